# revision 14
# baseline (speedup 1.0000x reference)
"""Trainium2 Bass kernel for a dense transformer block.

Sharding: 8-way SPMD, one (batch, half-sequence) shard of Tq=1024 query tokens
per core. Each core recomputes K/V for its whole batch (x/value rows are
host-rolled so the core's query tokens come first; softmax over keys is
permutation invariant). No collectives.

Layout: activations live transposed in SBUF as X^T [channel, token] so every
linear layer is matmul(lhsT=W[cin,cout], rhs=X^T) producing Y^T directly.
Dense GEMMs run as float32r (full-rate fp32 mode, free dim >= 256); attention
internals (Q/K/V, probs) are fp16 with fp32 PSUM accumulation. LN1 computes
stats in row-major layout (free-dim reduces) before transposing; LN2 computes
stats with ones-matmuls (cross-partition sums) giving partition-replicated
stats. Softmax skips the max-subtraction (scores are bounded), the row-sum
comes free from the exp activation's accum_out, and probabilities are
re-normalized with one 4x-mode DVE pass before PE-transposing A into A^T for
the AV matmul. Long-lived tensors share one SBUF pool with explicit tag-slot
reuse across phases (xn->OT->h, KT->x1, V->xq->xn2->x2).
"""

import sys

import numpy as np

if "/opt/trn_rl_repo" not in sys.path:
    sys.path.insert(0, "/opt/trn_rl_repo")

CFG_FULL = dict(
    Tq=1024, Tkv=2048, C=1024, H=16, D=64, HID=4096, NCLS=1000, EPS=1e-5,
    B=4, N=2048,
)


def _chunks(total, size):
    out = []
    s = 0
    while s < total:
        c = min(size, total - s)
        out.append((s, c))
        s += c
    return out


def emit_block(tc, out_ap, ins, cfg):
    """Emit the full transformer-block program for one core's shard."""
    import concourse.mybir as mybir
    from concourse.masks import make_identity

    nc = tc.nc
    f32 = mybir.dt.float32
    f16 = mybir.dt.float16
    f32r = mybir.dt.float32r
    AF = mybir.ActivationFunctionType
    OP = mybir.AluOpType
    AX = mybir.AxisListType

    Tq, Tkv, C, H, D = cfg["Tq"], cfg["Tkv"], cfg["C"], cfg["H"], cfg["D"]
    HID, NCLS, EPS = cfg["HID"], cfg["NCLS"], cfg["EPS"]
    P = 128
    CT = C // P
    KT = Tkv // P
    HT = HID // P
    VQ = max(1, KT // 4)      # V stored as 4 quarter tiles
    NVT = (KT + VQ - 1) // VQ
    HPK = min(HT, 4)          # h tiles packed 4 per slot
    NHT = (HT + HPK - 1) // HPK
    SCALE = C ** -0.5
    gelu_func = AF.Tanh if cfg.get("sim_gelu_tanh") else AF.Gelu

    assert H * D == C and D == 64 and C % P == 0 and Tkv % P == 0
    assert Tq % P == 0 and HID % P == 0 and H % 2 == 0 and CT % 2 == 0
    assert KT % VQ == 0 and HT % HPK == 0

    def r32(ap):
        return ap.bitcast(f32r)

    def pool(name, bufs=1, space="SBUF"):
        return tc.tile_pool(name=name, bufs=bufs, space=space)

    # ---------------- constants & params ----------------
    const_cm = pool("const")
    const_pool = const_cm.__enter__()

    ident32 = const_pool.tile([P, P], f32)
    make_identity(nc, ident32)
    ident16 = const_pool.tile([P, P], f16)
    make_identity(nc, ident16)
    ones128 = const_pool.tile([P, P], f32)
    nc.vector.memset(ones128, 1.0)
    ones_r = const_pool.tile([P, P], f32r)
    nc.vector.memset(ones_r.bitcast(f32), 1.0)
    eps_ap = const_pool.tile([P, 1], f32)
    nc.vector.memset(eps_ap, EPS)

    pp = {}
    with pool("ppps", bufs=2, space="PSUM") as psum_misc:
        def load_pp(vec_ap, n, key):
            nt = n // P
            ld = const_pool.tile([nt, P], f32, tag="pp_ld")
            nc.sync.dma_start(ld, vec_ap.rearrange("(a p) -> a p", p=P))
            ps = psum_misc.tile([P, nt], f32, tag="pp_ps")
            nc.tensor.matmul(ps, ld, ident32[:nt, :nt], is_transpose=True)
            dst = const_pool.tile([P, nt], f32, tag=f"pp_{key}")
            nc.vector.tensor_copy(dst, ps)
            pp[key] = dst

        for key in ["g1", "be1", "g2", "be2", "b_ap", "b2"]:
            load_pp(ins[key], C, key)
        load_pp(ins["b1"], HID, "b1")
    bout_sb = const_pool.tile([1, NCLS], f32r)
    nc.sync.dma_start(bout_sb, ins["b_out"][None, :].bitcast(f32r))

    # ---------------- long-lived arena ----------------
    arena_cm = pool("arena")
    arena = arena_cm.__enter__()

    def atile(slot, shape, dtype, name):
        return arena.tile(shape, dtype, tag=slot, name=name)

    # ---------------- helpers ----------------
    def emit_rows_to_T(rows_ap, T, dst_tiles, load_pool, tpsum):
        """DRAM [T, C] fp32 -> dst_tiles[ct][:, 0:T] = X^T tiles [128, T]."""
        ntt = T // P
        for tg in range(0, ntt, 4):
            gsz = min(4, ntt - tg)
            rows = []
            for j in range(gsz):
                r = load_pool.tile([P, C], f32, tag="rowload")
                nc.sync.dma_start(r, rows_ap[(tg + j) * P:(tg + j + 1) * P, :])
                rows.append(r)
            for ct in range(CT):
                ps = tpsum.tile([P, 4, P], f32, tag="tr_ps")
                for j in range(gsz):
                    nc.tensor.matmul(
                        ps[:, j, :], rows[j][:, ct * P:(ct + 1) * P], ident32,
                        is_transpose=True, start=(j == 0), stop=(j == gsz - 1),
                    )
                nc.vector.tensor_copy(
                    dst_tiles[ct][:, tg * P:(tg + gsz) * P],
                    ps[:, :gsz, :].rearrange("p g q -> p (g q)"),
                )

    def emit_layernorm_T(xT_tiles, T, g_pp, be_pp, dst_tiles, spool, stat_pool):
        """LayerNorm on transposed input (stats via ones-matmuls)."""
        for (toff, tsz) in _chunks(T, 512):
            s1 = spool.tile([P, tsz], f32, tag="ln_s1")
            s2 = spool.tile([P, tsz], f32, tag="ln_s2")
            for ct in range(CT):
                xc = xT_tiles[ct][:, toff:toff + tsz]
                nc.tensor.matmul(s1, ones128, xc,
                                 start=(ct == 0), stop=(ct == CT - 1))
                sq = stat_pool.tile([P, tsz], f32r, tag="ln_sq")
                nc.vector.tensor_tensor(sq, xc, xc, OP.mult)
                nc.tensor.matmul(s2, ones_r, sq,
                                 start=(ct == 0), stop=(ct == CT - 1))
            mu = stat_pool.tile([P, tsz], f32, tag="ln_mu")
            nc.vector.tensor_scalar_mul(mu, s1, 1.0 / C)
            m2 = stat_pool.tile([P, tsz], f32, tag="ln_m2")
            nc.vector.tensor_scalar_mul(m2, s2, 1.0 / C)
            musq = stat_pool.tile([P, tsz], f32, tag="ln_musq")
            nc.vector.tensor_tensor(musq, mu, mu, OP.mult)
            var = stat_pool.tile([P, tsz], f32, tag="ln_var")
            nc.vector.tensor_tensor(var, m2, musq, OP.subtract)
            A = stat_pool.tile([P, tsz], f32, tag="ln_A")
            emit_rsqrt(A, var, stat_pool, tsz)
            Bt = stat_pool.tile([P, tsz], f32, tag="ln_B")
            nc.vector.scalar_tensor_tensor(Bt, mu, -1.0, A, OP.mult, OP.mult)
            for ct in range(CT):
                xc = xT_tiles[ct][:, toff:toff + tsz]
                u = stat_pool.tile([P, tsz], f32, tag="ln_u")
                nc.vector.tensor_tensor(u, xc, A, OP.mult)
                nc.vector.tensor_tensor(u, u, Bt, OP.add)
                nc.vector.tensor_scalar(
                    dst_tiles[ct][:, toff:toff + tsz], u,
                    g_pp[:, ct:ct + 1], be_pp[:, ct:ct + 1], OP.mult, OP.add)

    def emit_rsqrt(dst, var, stat_pool, tsz):
        """dst = 1/sqrt(var+eps), with one Newton refinement."""
        std = stat_pool.tile([P, tsz], f32, tag="rs_std")
        nc.scalar.activation(std, var, AF.Sqrt, bias=eps_ap, scale=1.0)
        r0 = stat_pool.tile([P, tsz], f32, tag="rs_r0")
        nc.vector.reciprocal(r0, std)
        vpe = stat_pool.tile([P, tsz], f32, tag="rs_vpe")
        nc.vector.tensor_scalar_add(vpe, var, EPS)
        t0 = stat_pool.tile([P, tsz], f32, tag="rs_t0")
        nc.vector.tensor_tensor(t0, r0, r0, OP.mult)
        nc.vector.tensor_tensor(t0, t0, vpe, OP.mult)
        nc.vector.tensor_scalar(t0, t0, -0.5, 1.5, OP.mult, OP.add)
        nc.vector.tensor_tensor(dst, r0, t0, OP.mult)

    def load_w_ctp(w_ap_, m_total, wpool, tag):
        """[C, M] DRAM -> [128, CT, M] SBUF (row-tiled, fp32r)."""
        w_sb = wpool.tile([P, CT, m_total], f32r, tag=tag)
        nc.sync.dma_start(
            w_sb, w_ap_.rearrange("(ct p) m -> p ct m", p=P).bitcast(f32r))
        return w_sb

    # ====== Phase 1: rows of x -> per-token LN1 stats -> xn^T ======
    xn_tiles = [atile(f"a{ct}", [P, Tkv], f32r, f"xn{ct}") for ct in range(CT)]
    with pool("ld1", bufs=6) as load_pool, \
         pool("lnrow", bufs=2) as row_stat, \
         pool("trps1", bufs=3, space="PSUM") as tpsum:
        ntt = Tkv // P
        for tg in range(0, ntt, 4):
            gsz = min(4, ntt - tg)
            rows = []
            for j in range(gsz):
                r = load_pool.tile([P, C], f32, tag="rowload")
                nc.sync.dma_start(
                    r, ins["xs"][(tg + j) * P:(tg + j + 1) * P, :])
                s1 = row_stat.tile([P, 1], f32, tag="r_s1")
                nc.vector.reduce_sum(s1, r, axis=AX.X)
                sq = row_stat.tile([P, C], f32, tag="r_sq")
                s2 = row_stat.tile([P, 1], f32, tag="r_s2")
                nc.vector.scalar_tensor_tensor(
                    sq, r, 1.0, r, OP.bypass, OP.mult, accum_out=s2)
                mu = row_stat.tile([P, 1], f32, tag="r_mu")
                nc.vector.tensor_scalar_mul(mu, s1, 1.0 / C)
                m2 = row_stat.tile([P, 1], f32, tag="r_m2")
                nc.vector.tensor_scalar_mul(m2, s2, 1.0 / C)
                musq = row_stat.tile([P, 1], f32, tag="r_musq")
                nc.vector.tensor_tensor(musq, mu, mu, OP.mult)
                var = row_stat.tile([P, 1], f32, tag="r_var")
                nc.vector.tensor_tensor(var, m2, musq, OP.subtract)
                rstd = row_stat.tile([P, 1], f32, tag="r_rstd")
                emit_rsqrt(rstd, var, row_stat, 1)
                negmu = row_stat.tile([P, 1], f32, tag="r_negmu")
                nc.vector.tensor_scalar_mul(negmu, mu, -1.0)
                # rows <- (x - mu) * rstd   (token-wise, in place)
                nc.vector.tensor_scalar(r, r, negmu, rstd, OP.add, OP.mult)
                rows.append(r)
            for ct in range(CT):
                ps = tpsum.tile([P, 4, P], f32, tag="tr_ps")
                for j in range(gsz):
                    nc.tensor.matmul(
                        ps[:, j, :], rows[j][:, ct * P:(ct + 1) * P], ident32,
                        is_transpose=True, start=(j == 0), stop=(j == gsz - 1))
                # xn^T <- psum * g[c] + be[c]
                nc.vector.tensor_scalar(
                    xn_tiles[ct][:, tg * P:(tg + gsz) * P],
                    ps[:, :gsz, :].rearrange("p g q -> p (g q)"),
                    pp["g1"][:, ct:ct + 1], pp["be1"][:, ct:ct + 1],
                    OP.mult, OP.add)

    if cfg.get("stop_after") == 1:
        arena_cm.__exit__(None, None, None); const_cm.__exit__(None, None, None); return
    # ================= Phase 2: Q^T, K^T (fp16) =================
    QT = [atile(f"q{i}", [P, Tq], f16, f"QT{i}") for i in range(CT)]
    KTt = [atile(f"k{i}", [P, Tkv], f16, f"KT{i}") for i in range(CT)]
    with pool("wqk") as wpool, \
         pool("qkps", bufs=4, space="PSUM") as qk_psum:
        for (w_ap_, dst, T) in [(ins["wq"], QT, Tq), (ins["wk"], KTt, Tkv)]:
            w_sb = load_w_ctp(w_ap_, C, wpool, "wqk")
            for m in range(CT):
                for (toff, tsz) in _chunks(T, 512):
                    ps = qk_psum.tile([P, tsz], f32, tag="qk_ps")
                    for ct in range(CT):
                        nc.tensor.matmul(
                            ps, w_sb[:, ct, m * P:(m + 1) * P],
                            xn_tiles[ct][:, toff:toff + tsz],
                            start=(ct == 0), stop=(ct == CT - 1))
                    nc.vector.tensor_copy(dst[m][:, toff:toff + tsz], ps)

    if cfg.get("stop_after") == 2:
        arena_cm.__exit__(None, None, None); const_cm.__exit__(None, None, None); return
    # ================= Phase 3: V (fp16, [k, head, d]) =================
    Vq = [atile(f"v{i}", [P, VQ, H, D], f16, f"V{i}") for i in range(NVT)]
    with pool("wv") as wpool, \
         pool("vld", bufs=3) as vload, \
         pool("vt", bufs=2) as vt_pool, \
         pool("vtps", bufs=3, space="PSUM") as vt_psum, \
         pool("vps", bufs=3, space="PSUM") as v_psum:
        wv_sb = load_w_ctp(ins["wv"], C, wpool, "wv")
        for kt in range(KT):
            rows = vload.tile([P, C], f32, tag="vrow")
            nc.sync.dma_start(rows, ins["vals"][kt * P:(kt + 1) * P, :])
            vT_kt = vt_pool.tile([P, CT, P], f32r, tag="vTkt")
            for g0 in range(0, CT, 4):
                gsz = min(4, CT - g0)
                ps = vt_psum.tile([P, 4, P], f32, tag="vt_ps")
                for j in range(gsz):
                    nc.tensor.matmul(
                        ps[:, j, :], rows[:, (g0 + j) * P:(g0 + j + 1) * P],
                        ident32, is_transpose=True,
                        start=(j == 0), stop=(j == gsz - 1))
                nc.vector.tensor_copy(vT_kt[:, g0:g0 + gsz, :], ps[:, :gsz, :])
            for (noff, nsz) in _chunks(C, 512):
                vp = v_psum.tile([P, nsz], f32, tag="v_ps")
                for ct in range(CT):
                    nc.tensor.matmul(
                        vp, vT_kt[:, ct, :],
                        wv_sb[:, ct, noff:noff + nsz],
                        start=(ct == 0), stop=(ct == CT - 1))
                h0 = noff // D
                nc.vector.tensor_copy(
                    Vq[kt // VQ][:, kt % VQ, h0:h0 + nsz // D, :],
                    vp.rearrange("p (h d) -> p h d", d=D))

    if cfg.get("stop_after") == 3:
        arena_cm.__exit__(None, None, None); const_cm.__exit__(None, None, None); return
    # ================= Phase 4: attention =================
    OT = [atile(f"a{hp}", [D, 2, Tq], f32r, f"OT{hp}") for hp in range(H // 2)]
    with pool("expa", bufs=2) as expa_pool, \
         pool("at", bufs=2) as at_pool, \
         pool("attsmall", bufs=6) as small_pool, \
         pool("sps", bufs=2, space="PSUM") as s_psum, \
         pool("tps", bufs=2, space="PSUM") as t_psum, \
         pool("ops", bufs=2, space="PSUM") as o_psum:
        s_halves = _chunks(Tkv, 1024)
        for h in range(H):
            hp, hf = h // 2, h % 2
            qT_h = QT[hp][hf * D:(hf + 1) * D, :]
            kT_h = KTt[hp][hf * D:(hf + 1) * D, :]
            for (qoff, qsz) in _chunks(Tq, 512):
                AT = at_pool.tile([P, KT, qsz], f16, tag="AT")
                for qb in range(qsz // P):
                    qt0 = qoff + qb * P
                    expa = expa_pool.tile([P, Tkv], f16, tag="expa")
                    accs = small_pool.tile([P, len(s_halves)], f32,
                                           tag="accs")
                    for si, (koff, ksz) in enumerate(s_halves):
                        sp = s_psum.tile([P, ksz], f32, tag="s_ps")
                        for (k2, k2sz) in _chunks(ksz, 512):
                            nc.tensor.matmul(
                                sp[:, k2:k2 + k2sz],
                                qT_h[:, qt0:qt0 + P],
                                kT_h[:, koff + k2:koff + k2 + k2sz],
                                start=True, stop=True)
                        nc.scalar.activation(
                            expa[:, koff:koff + ksz], sp, AF.Exp,
                            bias=0.0, scale=SCALE,
                            accum_out=accs[:, si:si + 1])
                    den = small_pool.tile([P, 1], f32, tag="den")
                    nc.vector.reduce_sum(den, accs, axis=AX.X)
                    rec = small_pool.tile([P, 1], f32, tag="rec")
                    nc.vector.reciprocal(rec, den)
                    ean = expa_pool.tile([P, Tkv], f16, tag="ean")
                    nc.vector.tensor_scalar_mul(ean, expa, rec)
                    for g0 in range(0, KT, 8):
                        gsz = min(8, KT - g0)
                        tp = t_psum.tile([P, 8, P], f16, tag="t_ps")
                        for j in range(gsz):
                            kb = g0 + j
                            nc.tensor.matmul(
                                tp[:, j, :], ean[:, kb * P:(kb + 1) * P],
                                ident16, is_transpose=True,
                                start=(j == 0), stop=(j == gsz - 1))
                        dst = AT[:, g0:g0 + gsz, qb * P:(qb + 1) * P]
                        if (g0 // 8 + qb) % 2 == 0:
                            nc.vector.tensor_copy(dst, tp[:, :gsz, :])
                        else:
                            nc.scalar.copy(dst, tp[:, :gsz, :])
                op = o_psum.tile([D, qsz], f32, tag="o_ps")
                for kt in range(KT):
                    nc.tensor.matmul(
                        op, Vq[kt // VQ][:, kt % VQ, h, :], AT[:, kt, :],
                        start=(kt == 0), stop=(kt == KT - 1))
                nc.vector.tensor_copy(OT[hp][:, hf, qoff:qoff + qsz], op)

    if cfg.get("stop_after") == 4:
        arena_cm.__exit__(None, None, None); const_cm.__exit__(None, None, None); return
    # ================= Phase 5: attn proj + residual 1 =================
    x1 = [atile(f"k{ct}", [P, Tq], f32, f"x1_{ct}") for ct in range(CT)]
    with pool("xqld", bufs=6) as load_pool, \
         pool("wap", bufs=2) as wap_pool, \
         pool("xqps", bufs=3, space="PSUM") as tpsum, \
         pool("apps", bufs=4, space="PSUM") as ap_psum:
        xq_pairs = [atile(f"v{i}", [P, 2, Tq], f32, f"xq{i}")
                    for i in range(CT // 2)]
        xqT = [xq_pairs[ct // 2][:, ct % 2, :] for ct in range(CT)]
        emit_rows_to_T(ins["xs"][:Tq, :], Tq, xqT, load_pool, tpsum)
        wap_r = ins["w_ap"].rearrange("(a p) m -> p a m", p=D)
        for ct in range(CT):
            wap_ct = wap_pool.tile([D, H, P], f32r, tag="wap_ct")
            nc.sync.dma_start(wap_ct, wap_r[:, :, ct * P:(ct + 1) * P].bitcast(f32r))
            for (toff, tsz) in _chunks(Tq, 512):
                ps = ap_psum.tile([P, tsz], f32, tag="ap_ps")
                for ht in range(H):
                    nc.tensor.matmul(
                        ps, wap_ct[:, ht, :],
                        OT[ht // 2][:, ht % 2, toff:toff + tsz],
                        start=(ht == 0), stop=(ht == H - 1))
                nc.vector.scalar_tensor_tensor(
                    x1[ct][:, toff:toff + tsz], ps, pp["b_ap"][:, ct:ct + 1],
                    xqT[ct][:, toff:toff + tsz], OP.add, OP.add)

    if cfg.get("stop_after") == 5:
        arena_cm.__exit__(None, None, None); const_cm.__exit__(None, None, None); return
    # ============ Phase 6+7: LN2, fc1 + gelu -> h^T fp16 ============
    with pool("ln2ps", bufs=2, space="PSUM") as ln_spool, \
         pool("ln2stat", bufs=1) as ln_stat, \
         pool("w1", bufs=3) as w1_pool, \
         pool("f1ps", bufs=4, space="PSUM") as f1_psum:
        xn2_pairs = [atile(f"v{i}", [P, 2, Tq], f32r, f"xn2_{i}")
                     for i in range(CT // 2)]
        xn2 = [xn2_pairs[ct // 2][:, ct % 2, :] for ct in range(CT)]
        emit_layernorm_T(x1, Tq, pp["g2"], pp["be2"], xn2, ln_spool, ln_stat)
        hT_g = [atile(f"a{g}", [P, HPK, Tq], f16, f"hT{g}")
                for g in range(NHT)]
        hT = [hT_g[m // HPK][:, m % HPK, :] for m in range(HT)]
        w1r = ins["w1"].rearrange("(ct p) m -> p ct m", p=P)
        for m in range(HT):
            w1_cb = w1_pool.tile([P, CT, P], f32r, tag="w1cb")
            nc.sync.dma_start(w1_cb, w1r[:, :, m * P:(m + 1) * P].bitcast(f32r))
            for (toff, tsz) in _chunks(Tq, 512):
                ps = f1_psum.tile([P, tsz], f32, tag="f1_ps")
                for ct in range(CT):
                    nc.tensor.matmul(
                        ps, w1_cb[:, ct, :],
                        xn2[ct][:, toff:toff + tsz],
                        start=(ct == 0), stop=(ct == CT - 1))
                nc.scalar.activation(
                    hT[m][:, toff:toff + tsz], ps, gelu_func,
                    bias=pp["b1"][:, m:m + 1], scale=1.0)

    if cfg.get("stop_after") == 7:
        arena_cm.__exit__(None, None, None); const_cm.__exit__(None, None, None); return
    # ================= Phase 8: fc2 + residual 2 =================
    x2_pairs = [atile(f"v{i}", [P, 2, Tq], f32r, f"x2_{i}")
                for i in range(CT // 2)]
    x2 = [x2_pairs[ct // 2][:, ct % 2, :] for ct in range(CT)]
    with pool("w2f", bufs=1) as w2f_pool, \
         pool("w2h", bufs=2) as w2h_pool, \
         pool("f2ps", bufs=4, space="PSUM") as f2_psum:
        w2r = ins["w2"].rearrange("(ht p) c -> p ht c", p=P)
        for ct in range(CT):
            w2_f32 = w2f_pool.tile([P, HT, P], f32, tag="w2f32")
            nc.sync.dma_start(w2_f32, w2r[:, :, ct * P:(ct + 1) * P])
            w2_f16 = w2h_pool.tile([P, HT, P], f16, tag="w2f16")
            nc.vector.tensor_copy(w2_f16, w2_f32)
            for (toff, tsz) in _chunks(Tq, 512):
                ps = f2_psum.tile([P, tsz], f32, tag="f2_ps")
                for ht in range(HT):
                    nc.tensor.matmul(
                        ps, w2_f16[:, ht, :], hT[ht][:, toff:toff + tsz],
                        start=(ht == 0), stop=(ht == HT - 1))
                nc.vector.scalar_tensor_tensor(
                    x2[ct][:, toff:toff + tsz], ps, pp["b2"][:, ct:ct + 1],
                    x1[ct][:, toff:toff + tsz], OP.add, OP.add)

    if cfg.get("stop_after") == 8:
        arena_cm.__exit__(None, None, None); const_cm.__exit__(None, None, None); return
    # ================= Phase 9: out proj + softmax =================
    with pool("wout") as wpool, \
         pool("smax", bufs=3) as sm_pool, \
         pool("smsmall", bufs=6) as sms_pool, \
         pool("outps", bufs=4, space="PSUM") as out_psum:
        wout_sb = load_w_ctp(ins["w_out"], NCLS, wpool, "wout")
        n_chunks = _chunks(NCLS, 500)
        for tt in range(Tq // P):
            pss = []
            for (noff, nsz) in n_chunks:
                ps = out_psum.tile([P, nsz], f32, tag="out_ps")
                for ct in range(CT):
                    nc.tensor.matmul(
                        ps, x2[ct][:, tt * P:(tt + 1) * P],
                        wout_sb[:, ct, noff:noff + nsz],
                        start=(ct == 0), stop=False)
                nc.tensor.matmul(
                    ps, ones_r[0:1, :],
                    bout_sb[0:1, noff:noff + nsz],
                    start=False, stop=True)
                pss.append(ps)
            mx = sms_pool.tile([P, len(n_chunks)], f32, tag="sm_mx")
            for i, ps in enumerate(pss):
                nc.vector.reduce_max(mx[:, i:i + 1], ps, axis=AX.X)
            m = sms_pool.tile([P, 1], f32, tag="sm_m")
            nc.vector.reduce_max(m, mx, axis=AX.X)
            negm = sms_pool.tile([P, 1], f32, tag="sm_negm")
            nc.vector.tensor_scalar_mul(negm, m, -1.0)
            esb = sm_pool.tile([P, NCLS], f32, tag="sm_e")
            accs = sms_pool.tile([P, len(n_chunks)], f32, tag="sm_acc")
            for i, ((noff, nsz), ps) in enumerate(zip(n_chunks, pss)):
                nc.scalar.activation(
                    esb[:, noff:noff + nsz], ps, AF.Exp,
                    bias=negm, scale=1.0, accum_out=accs[:, i:i + 1])
            s = sms_pool.tile([P, 1], f32, tag="sm_s")
            nc.vector.reduce_sum(s, accs, axis=AX.X)
            rec = sms_pool.tile([P, 1], f32, tag="sm_rec")
            nc.vector.reciprocal(rec, s)
            nc.vector.tensor_scalar_mul(esb, esb, rec)
            nc.sync.dma_start(out_ap[tt * P:(tt + 1) * P, :], esb)

    arena_cm.__exit__(None, None, None)
    const_cm.__exit__(None, None, None)


# ======================= host entry =======================

_IN_NAMES = ["xs", "vals", "wq", "wk", "wv", "w_ap", "b_ap", "g1", "be1",
             "g2", "be2", "w1", "b1", "w2", "b2", "w_out", "b_out"]


def _build_nc(cfg):
    import concourse.bacc as bacc
    import concourse.mybir as mybir
    import concourse.tile as tile

    Tq, Tkv, C = cfg["Tq"], cfg["Tkv"], cfg["C"]
    HID, NCLS = cfg["HID"], cfg["NCLS"]
    shapes = dict(
        xs=[Tkv, C], vals=[Tkv, C], wq=[C, C], wk=[C, C], wv=[C, C],
        w_ap=[C, C], b_ap=[C], g1=[C], be1=[C], g2=[C], be2=[C],
        w1=[C, HID], b1=[HID], w2=[HID, C], b2=[C],
        w_out=[C, NCLS], b_out=[NCLS],
    )
    nc = bacc.Bacc("TRN2", target_bir_lowering=False, debug=False)
    ins = {k: nc.dram_tensor(k, shapes[k], mybir.dt.float32,
                             kind="ExternalInput").ap()
           for k in _IN_NAMES}
    out_ap = nc.dram_tensor("out", [Tq, NCLS], mybir.dt.float32,
                            kind="ExternalOutput").ap()
    with tile.TileContext(nc) as tc:
        emit_block(tc, out_ap, ins, cfg)
    nc.finalize()
    return nc


_NC_CACHE = {}


def kernel(**inputs) -> np.ndarray:
    from concourse.bass_utils import run_bass_kernel_spmd

    cfg = CFG_FULL
    B, N = cfg["B"], cfg["N"]
    Tq, NCLS = cfg["Tq"], cfg["NCLS"]
    n_cores = 8
    halves = N // Tq  # 2

    if "full" not in _NC_CACHE:
        _NC_CACHE["full"] = _build_nc(cfg)
    nc = _NC_CACHE["full"]

    x = np.ascontiguousarray(np.asarray(inputs["x"], dtype=np.float32))
    value = np.ascontiguousarray(np.asarray(inputs["value"], dtype=np.float32))
    shared = {k: np.ascontiguousarray(np.asarray(inputs[k], dtype=np.float32))
              for k in _IN_NAMES if k not in ("xs", "vals")}

    in_maps = []
    for core in range(n_cores):
        b, hf = core // halves, core % halves
        m = dict(shared)
        m["xs"] = np.ascontiguousarray(np.roll(x[b], -hf * Tq, axis=0))
        m["vals"] = np.ascontiguousarray(np.roll(value[b], -hf * Tq, axis=0))
        in_maps.append(m)

    res = run_bass_kernel_spmd(nc, in_maps, core_ids=list(range(n_cores)))
    out = np.empty((B, N, NCLS), dtype=np.float32)
    for core in range(n_cores):
        b, hf = core // halves, core % halves
        out[b, hf * Tq:(hf + 1) * Tq, :] = res.results[core]["out"]
    return out


# revision 21
# speedup vs baseline: 62.1423x; 62.1423x over previous
"""Trainium2 Bass kernel for a dense transformer block.

Sharding: 8-way SPMD, one (batch, half-sequence) shard of Tq=1024 query tokens
per core. Each core recomputes K/V for its whole batch (x/value rows are
host-rolled so the core's query tokens come first; softmax over keys is
permutation invariant). No collectives.

Layout: activations live transposed in SBUF as X^T [channel, token] so every
linear layer is matmul(lhsT=W[cin,cout], rhs=X^T) producing Y^T directly.
Dense GEMMs run as float32r (full-rate fp32 mode, free dim >= 256); attention
internals (Q/K/V, probs) are fp16 with fp32 PSUM accumulation. LN1 computes
stats in row-major layout (free-dim reduces) before transposing; LN2 computes
stats with ones-matmuls (cross-partition sums) giving partition-replicated
stats. Softmax skips the max-subtraction (scores are bounded), the row-sum
comes free from the exp activation's accum_out, and probabilities are
re-normalized with one 4x-mode DVE pass before PE-transposing A into A^T for
the AV matmul. Long-lived tensors share one SBUF pool with explicit tag-slot
reuse across phases (xn->OT->h, KT->x1, V->xq->xn2->x2).
"""

import sys

import numpy as np

if "/opt/trn_rl_repo" not in sys.path:
    sys.path.insert(0, "/opt/trn_rl_repo")

CFG_FULL = dict(
    Tq=1024, Tkv=2048, C=1024, H=16, D=64, HID=4096, NCLS=1000, EPS=1e-5,
    B=4, N=2048,
)


def _chunks(total, size):
    out = []
    s = 0
    while s < total:
        c = min(size, total - s)
        out.append((s, c))
        s += c
    return out


def emit_block(tc, out_ap, ins, cfg):
    """Emit the full transformer-block program for one core's shard."""
    import concourse.mybir as mybir
    from concourse.masks import make_identity

    nc = tc.nc
    f32 = mybir.dt.float32
    f16 = mybir.dt.float16
    f32r = mybir.dt.float32r
    AF = mybir.ActivationFunctionType
    OP = mybir.AluOpType
    AX = mybir.AxisListType

    Tq, Tkv, C, H, D = cfg["Tq"], cfg["Tkv"], cfg["C"], cfg["H"], cfg["D"]
    HID, NCLS, EPS = cfg["HID"], cfg["NCLS"], cfg["EPS"]
    P = 128
    CT = C // P
    KT = Tkv // P
    HT = HID // P
    VQ = max(1, KT // 4)      # V stored as 4 quarter tiles
    NVT = (KT + VQ - 1) // VQ
    HPK = min(HT, 4)          # h tiles packed 4 per slot
    NHT = (HT + HPK - 1) // HPK
    SCALE = C ** -0.5
    gelu_func = AF.Tanh if cfg.get("sim_gelu_tanh") else AF.Gelu

    assert H * D == C and D == 64 and C % P == 0 and Tkv % P == 0
    assert Tq % P == 0 and HID % P == 0 and H % 2 == 0 and CT % 2 == 0
    assert KT % VQ == 0 and HT % HPK == 0

    def r32(ap):
        return ap.bitcast(f32r)

    def pool(name, bufs=1, space="SBUF"):
        return tc.tile_pool(name=name, bufs=bufs, space=space)

    # ---------------- constants & params ----------------
    const_cm = pool("const")
    const_pool = const_cm.__enter__()

    ident32 = const_pool.tile([P, P], f32)
    make_identity(nc, ident32)
    ones128 = const_pool.tile([P, P], f32)
    nc.vector.memset(ones128, 1.0)
    ones_r = const_pool.tile([P, P], f32r)
    nc.vector.memset(ones_r.bitcast(f32), 1.0)
    eps_ap = const_pool.tile([P, 1], f32)
    nc.vector.memset(eps_ap, EPS)

    pp = {}
    with pool("ppps", bufs=2, space="PSUM") as psum_misc:
        def load_pp(vec_ap, n, key):
            nt = n // P
            ld = const_pool.tile([nt, P], f32, tag="pp_ld")
            nc.sync.dma_start(ld, vec_ap.rearrange("(a p) -> a p", p=P))
            ps = psum_misc.tile([P, nt], f32, tag="pp_ps")
            nc.tensor.matmul(ps, ld, ident32[:nt, :nt], is_transpose=True)
            dst = const_pool.tile([P, nt], f32, tag=f"pp_{key}")
            nc.vector.tensor_copy(dst, ps)
            pp[key] = dst

        for key in ["g1", "be1", "g2", "be2", "b_ap", "b2"]:
            load_pp(ins[key], C, key)
        load_pp(ins["b1"], HID, "b1")
    bout_sb = const_pool.tile([1, NCLS], f32r)
    nc.sync.dma_start(bout_sb, ins["b_out"][None, :].bitcast(f32r))

    # ---------------- long-lived arena ----------------
    arena_cm = pool("arena")
    arena = arena_cm.__enter__()

    def atile(slot, shape, dtype, name):
        return arena.tile(shape, dtype, tag=slot, name=name)

    # ---------------- helpers ----------------
    def emit_rows_to_T(rows_ap, T, dst_tiles, load_pool, tpsum):
        """DRAM [T, C] fp32 -> dst_tiles[ct][:, 0:T] = X^T tiles [128, T]."""
        ntt = T // P
        for tg in range(0, ntt, 4):
            gsz = min(4, ntt - tg)
            rows = []
            for j in range(gsz):
                r = load_pool.tile([P, C], f32, tag="rowload")
                nc.sync.dma_start(r, rows_ap[(tg + j) * P:(tg + j + 1) * P, :])
                rows.append(r)
            for ct in range(CT):
                ps = tpsum.tile([P, 4, P], f32, tag="tr_ps")
                for j in range(gsz):
                    nc.tensor.matmul(
                        ps[:, j, :], rows[j][:, ct * P:(ct + 1) * P], ident32,
                        is_transpose=True, start=(j == 0), stop=(j == gsz - 1),
                    )
                nc.vector.tensor_copy(
                    dst_tiles[ct][:, tg * P:(tg + gsz) * P],
                    ps[:, :gsz, :].rearrange("p g q -> p (g q)"),
                )

    def emit_layernorm_T(xT_tiles, T, g_pp, be_pp, dst_tiles, spool, stat_pool):
        """LayerNorm on transposed input (stats via ones-matmuls)."""
        for (toff, tsz) in _chunks(T, 512):
            s1 = spool.tile([P, tsz], f32, tag="ln_s1")
            s2 = spool.tile([P, tsz], f32, tag="ln_s2")
            for ct in range(CT):
                xc = xT_tiles[ct][:, toff:toff + tsz]
                nc.tensor.matmul(s1, ones128, xc,
                                 start=(ct == 0), stop=(ct == CT - 1))
                sq = stat_pool.tile([P, tsz], f32r, tag="ln_sq")
                nc.vector.tensor_tensor(sq, xc, xc, OP.mult)
                nc.tensor.matmul(s2, ones_r, sq,
                                 start=(ct == 0), stop=(ct == CT - 1))
            mu = stat_pool.tile([P, tsz], f32, tag="ln_mu")
            nc.vector.tensor_scalar_mul(mu, s1, 1.0 / C)
            m2 = stat_pool.tile([P, tsz], f32, tag="ln_m2")
            nc.vector.tensor_scalar_mul(m2, s2, 1.0 / C)
            musq = stat_pool.tile([P, tsz], f32, tag="ln_musq")
            nc.vector.tensor_tensor(musq, mu, mu, OP.mult)
            var = stat_pool.tile([P, tsz], f32, tag="ln_var")
            nc.vector.tensor_tensor(var, m2, musq, OP.subtract)
            A = stat_pool.tile([P, tsz], f32, tag="ln_A")
            emit_rsqrt(A, var, stat_pool, tsz)
            Bt = stat_pool.tile([P, tsz], f32, tag="ln_B")
            nc.vector.scalar_tensor_tensor(Bt, mu, -1.0, A, OP.mult, OP.mult)
            for ct in range(CT):
                xc = xT_tiles[ct][:, toff:toff + tsz]
                u = stat_pool.tile([P, tsz], f32, tag="ln_u")
                nc.vector.tensor_tensor(u, xc, A, OP.mult)
                nc.vector.tensor_tensor(u, u, Bt, OP.add)
                nc.vector.tensor_scalar(
                    dst_tiles[ct][:, toff:toff + tsz], u,
                    g_pp[:, ct:ct + 1], be_pp[:, ct:ct + 1], OP.mult, OP.add)

    def emit_rsqrt(dst, var, stat_pool, tsz):
        """dst = 1/sqrt(var+eps), with one Newton refinement."""
        std = stat_pool.tile([P, tsz], f32, tag="rs_std")
        nc.scalar.activation(std, var, AF.Sqrt, bias=eps_ap, scale=1.0)
        r0 = stat_pool.tile([P, tsz], f32, tag="rs_r0")
        nc.vector.reciprocal(r0, std)
        vpe = stat_pool.tile([P, tsz], f32, tag="rs_vpe")
        nc.vector.tensor_scalar_add(vpe, var, EPS)
        t0 = stat_pool.tile([P, tsz], f32, tag="rs_t0")
        nc.vector.tensor_tensor(t0, r0, r0, OP.mult)
        nc.vector.tensor_tensor(t0, t0, vpe, OP.mult)
        nc.vector.tensor_scalar(t0, t0, -0.5, 1.5, OP.mult, OP.add)
        nc.vector.tensor_tensor(dst, r0, t0, OP.mult)

    def load_w_ctp(w_ap_, m_total, wpool, tag):
        """[C, M] DRAM -> [128, CT, M] SBUF (row-tiled, fp32r)."""
        w_sb = wpool.tile([P, CT, m_total], f32r, tag=tag)
        nc.sync.dma_start(
            w_sb, w_ap_.rearrange("(ct p) m -> p ct m", p=P).bitcast(f32r))
        return w_sb

    # ====== Phase 1: rows of x -> per-token LN1 stats -> xn^T ======
    xn_tiles = [atile(f"a{ct}", [P, Tkv], f32r, f"xn{ct}") for ct in range(CT)]
    with pool("ld1", bufs=6) as load_pool, \
         pool("lnrow", bufs=2) as row_stat, \
         pool("trps1", bufs=3, space="PSUM") as tpsum:
        ntt = Tkv // P
        for tg in range(0, ntt, 4):
            gsz = min(4, ntt - tg)
            rows = []
            for j in range(gsz):
                r = load_pool.tile([P, C], f32, tag="rowload")
                nc.sync.dma_start(
                    r, ins["xs"][(tg + j) * P:(tg + j + 1) * P, :])
                s1 = row_stat.tile([P, 1], f32, tag="r_s1")
                nc.vector.reduce_sum(s1, r, axis=AX.X)
                sq = row_stat.tile([P, C], f32, tag="r_sq")
                s2 = row_stat.tile([P, 1], f32, tag="r_s2")
                nc.vector.scalar_tensor_tensor(
                    sq, r, 1.0, r, OP.bypass, OP.mult, accum_out=s2)
                mu = row_stat.tile([P, 1], f32, tag="r_mu")
                nc.vector.tensor_scalar_mul(mu, s1, 1.0 / C)
                m2 = row_stat.tile([P, 1], f32, tag="r_m2")
                nc.vector.tensor_scalar_mul(m2, s2, 1.0 / C)
                musq = row_stat.tile([P, 1], f32, tag="r_musq")
                nc.vector.tensor_tensor(musq, mu, mu, OP.mult)
                var = row_stat.tile([P, 1], f32, tag="r_var")
                nc.vector.tensor_tensor(var, m2, musq, OP.subtract)
                rstd = row_stat.tile([P, 1], f32, tag="r_rstd")
                emit_rsqrt(rstd, var, row_stat, 1)
                negmu = row_stat.tile([P, 1], f32, tag="r_negmu")
                nc.vector.tensor_scalar_mul(negmu, mu, -1.0)
                # rows <- (x - mu) * rstd   (token-wise, in place)
                nc.vector.tensor_scalar(r, r, negmu, rstd, OP.add, OP.mult)
                rows.append(r)
            for ct in range(CT):
                ps = tpsum.tile([P, 4, P], f32, tag="tr_ps")
                for j in range(gsz):
                    nc.tensor.matmul(
                        ps[:, j, :], rows[j][:, ct * P:(ct + 1) * P], ident32,
                        is_transpose=True, start=(j == 0), stop=(j == gsz - 1))
                # xn^T <- psum * g[c] + be[c]
                nc.vector.tensor_scalar(
                    xn_tiles[ct][:, tg * P:(tg + gsz) * P],
                    ps[:, :gsz, :].rearrange("p g q -> p (g q)"),
                    pp["g1"][:, ct:ct + 1], pp["be1"][:, ct:ct + 1],
                    OP.mult, OP.add)

    if cfg.get("stop_after") == 1:
        arena_cm.__exit__(None, None, None); const_cm.__exit__(None, None, None); return
    # ================= Phase 2: Q^T, K^T (fp16) =================
    QT = [atile(f"q{i}", [P, Tq], f16, f"QT{i}") for i in range(CT)]
    KTt = [atile(f"k{i}", [P, Tkv], f16, f"KT{i}") for i in range(CT)]
    with pool("wqk") as wpool, \
         pool("qkps", bufs=4, space="PSUM") as qk_psum:
        for (w_ap_, dst, T) in [(ins["wq"], QT, Tq), (ins["wk"], KTt, Tkv)]:
            w_sb = load_w_ctp(w_ap_, C, wpool, "wqk")
            for m in range(CT):
                for (toff, tsz) in _chunks(T, 512):
                    ps = qk_psum.tile([P, tsz], f32, tag="qk_ps")
                    for ct in range(CT):
                        nc.tensor.matmul(
                            ps, w_sb[:, ct, m * P:(m + 1) * P],
                            xn_tiles[ct][:, toff:toff + tsz],
                            start=(ct == 0), stop=(ct == CT - 1))
                    nc.vector.tensor_copy(dst[m][:, toff:toff + tsz], ps)

    if cfg.get("stop_after") == 2:
        arena_cm.__exit__(None, None, None); const_cm.__exit__(None, None, None); return
    # ========== Phase 3: V (fp16, [k, head, d+ones]) ==========
    # Column D of each head's 65-wide slot is 1.0 so the AV matmul's output
    # row 64 accumulates the softmax denominator for free.
    DA = D + 1
    Vq = [atile(f"v{i}", [P, VQ, H, DA], f16, f"V{i}") for i in range(NVT)]
    for vq in Vq:
        nc.vector.memset(vq, 1.0)
    with pool("wv") as wpool, \
         pool("vld", bufs=3) as vload, \
         pool("vt", bufs=2) as vt_pool, \
         pool("vtps", bufs=3, space="PSUM") as vt_psum, \
         pool("vps", bufs=3, space="PSUM") as v_psum:
        wv_sb = load_w_ctp(ins["wv"], C, wpool, "wv")
        for kt in range(KT):
            rows = vload.tile([P, C], f32, tag="vrow")
            nc.sync.dma_start(rows, ins["vals"][kt * P:(kt + 1) * P, :])
            vT_kt = vt_pool.tile([P, CT, P], f32r, tag="vTkt")
            for g0 in range(0, CT, 4):
                gsz = min(4, CT - g0)
                ps = vt_psum.tile([P, 4, P], f32, tag="vt_ps")
                for j in range(gsz):
                    nc.tensor.matmul(
                        ps[:, j, :], rows[:, (g0 + j) * P:(g0 + j + 1) * P],
                        ident32, is_transpose=True,
                        start=(j == 0), stop=(j == gsz - 1))
                nc.vector.tensor_copy(vT_kt[:, g0:g0 + gsz, :], ps[:, :gsz, :])
            for (noff, nsz) in _chunks(C, 512):
                vp = v_psum.tile([P, nsz], f32, tag="v_ps")
                for ct in range(CT):
                    nc.tensor.matmul(
                        vp, vT_kt[:, ct, :],
                        wv_sb[:, ct, noff:noff + nsz],
                        start=(ct == 0), stop=(ct == CT - 1))
                h0 = noff // D
                nc.vector.tensor_copy(
                    Vq[kt // VQ][:, kt % VQ, h0:h0 + nsz // D, 0:D],
                    vp.rearrange("p (h d) -> p h d", d=D))

    if cfg.get("stop_after") == 3:
        arena_cm.__exit__(None, None, None); const_cm.__exit__(None, None, None); return
    # ================= Phase 4: attention (S^T dataflow) =================
    # S^T[k, q] = matmul(lhsT=K^T chunk, rhs=Q^T) puts keys on partitions, so
    # exp(S^T) is directly the AV matmul's moving operand — no PE transposes
    # and no PSUM->SBUF prob copies. AV's lhsT is [V_h | ones] so PSUM row 64
    # is the softmax denominator; 1/den is broadcast across partitions with a
    # K=1 ones-matmul and folded into the O^T copy.
    OT = [atile(f"a{hp}", [D, 2, Tq], f32r, f"OT{hp}") for hp in range(H // 2)]
    with pool("es", bufs=2) as es_pool, \
         pool("attsmall", bufs=4) as small_pool, \
         pool("sps", bufs=2, space="PSUM") as s_psum, \
         pool("ops", bufs=2, space="PSUM") as o_psum, \
         pool("bcps", bufs=2, space="PSUM") as bc_psum:
        for h in range(H):
            hp, hf = h // 2, h % 2
            qT_h = QT[hp][hf * D:(hf + 1) * D, :]
            kT_h = KTt[hp][hf * D:(hf + 1) * D, :]
            for (qoff, qsz) in _chunks(Tq, 512):
                ES = es_pool.tile([P, KT, qsz], f16, tag="ES")
                for kt2 in range(0, KT, 2):
                    kn = min(2, KT - kt2)
                    sp = s_psum.tile([P, 2, qsz], f32, tag="s_ps")
                    one_bank = qsz * 4 * kn <= 2048
                    for j in range(kn):
                        nc.tensor.matmul(
                            sp[:, j, :],
                            kT_h[:, (kt2 + j) * P:(kt2 + j + 1) * P],
                            qT_h[:, qoff:qoff + qsz],
                            start=(j == 0 if one_bank else True),
                            stop=(j == kn - 1 if one_bank else True))
                    nc.scalar.activation(
                        ES[:, kt2:kt2 + kn, :], sp[:, :kn, :], AF.Exp,
                        bias=0.0, scale=SCALE)
                op = o_psum.tile([DA, qsz], f32, tag="o_ps")
                for kt in range(KT):
                    nc.tensor.matmul(
                        op, Vq[kt // VQ][:, kt % VQ, h, :], ES[:, kt, :],
                        start=(kt == 0), stop=(kt == KT - 1))
                rec = small_pool.tile([DA, qsz], f32, tag="rec")
                nc.vector.reciprocal(rec[D:DA, :], op[D:DA, :])
                recr = small_pool.tile([DA, qsz], f32r, tag="recr")
                nc.vector.tensor_copy(recr[D:DA, :], rec[D:DA, :])
                bc = bc_psum.tile([D, qsz], f32, tag="bc")
                nc.tensor.matmul(bc, ones_r[D:D + 1, 0:D],
                                 recr[D:DA, :], start=True, stop=True)
                bc_sb = small_pool.tile([D, qsz], f32, tag="bc_sb")
                nc.scalar.copy(bc_sb, bc)
                nc.vector.tensor_tensor(
                    OT[hp][:, hf, qoff:qoff + qsz], op[0:D, :], bc_sb, OP.mult)

    if cfg.get("stop_after") == 4:
        arena_cm.__exit__(None, None, None); const_cm.__exit__(None, None, None); return
    # ================= Phase 5: attn proj + residual 1 =================
    x1 = [atile(f"k{ct}", [P, Tq], f32, f"x1_{ct}") for ct in range(CT)]
    with pool("xqld", bufs=6) as load_pool, \
         pool("wap", bufs=2) as wap_pool, \
         pool("xqps", bufs=3, space="PSUM") as tpsum, \
         pool("apps", bufs=4, space="PSUM") as ap_psum:
        xq_pairs = [atile(f"v{i}", [P, 2, Tq], f32, f"xq{i}")
                    for i in range(CT // 2)]
        xqT = [xq_pairs[ct // 2][:, ct % 2, :] for ct in range(CT)]
        emit_rows_to_T(ins["xs"][:Tq, :], Tq, xqT, load_pool, tpsum)
        wap_r = ins["w_ap"].rearrange("(a p) m -> p a m", p=D)
        for ct in range(CT):
            wap_ct = wap_pool.tile([D, H, P], f32r, tag="wap_ct")
            nc.gpsimd.dma_start(wap_ct, wap_r[:, :, ct * P:(ct + 1) * P].bitcast(f32r))
            for (toff, tsz) in _chunks(Tq, 512):
                ps = ap_psum.tile([P, tsz], f32, tag="ap_ps")
                for ht in range(H):
                    nc.tensor.matmul(
                        ps, wap_ct[:, ht, :],
                        OT[ht // 2][:, ht % 2, toff:toff + tsz],
                        start=(ht == 0), stop=(ht == H - 1))
                nc.vector.scalar_tensor_tensor(
                    x1[ct][:, toff:toff + tsz], ps, pp["b_ap"][:, ct:ct + 1],
                    xqT[ct][:, toff:toff + tsz], OP.add, OP.add)

    if cfg.get("stop_after") == 5:
        arena_cm.__exit__(None, None, None); const_cm.__exit__(None, None, None); return
    # ============ Phase 6+7: LN2, fc1 + gelu -> h^T fp16 ============
    with pool("ln2ps", bufs=2, space="PSUM") as ln_spool, \
         pool("ln2stat", bufs=1) as ln_stat, \
         pool("w1", bufs=2) as w1_pool, \
         pool("f1ps", bufs=4, space="PSUM") as f1_psum:
        xn2_pairs = [atile(f"v{i}", [P, 2, Tq], f32r, f"xn2_{i}")
                     for i in range(CT // 2)]
        xn2 = [xn2_pairs[ct // 2][:, ct % 2, :] for ct in range(CT)]
        emit_layernorm_T(x1, Tq, pp["g2"], pp["be2"], xn2, ln_spool, ln_stat)
        hT_g = [atile(f"a{g}", [P, HPK, Tq], f16, f"hT{g}")
                for g in range(NHT)]
        hT = [hT_g[m // HPK][:, m % HPK, :] for m in range(HT)]
        w1r = ins["w1"].rearrange("(ct p) m -> p ct m", p=P)
        # stream w1 in column chunks of 4 m-tiles (fewer, fatter descriptors)
        W1CH = min(4 * P, HID)
        for (moff, msz) in _chunks(HID, W1CH):
            w1_cb = w1_pool.tile([P, CT, W1CH], f32r, tag="w1cb")
            nc.gpsimd.dma_start(
                w1_cb[:, :, :msz],
                w1r[:, :, moff:moff + msz].bitcast(f32r))
            for mi in range(msz // P):
                m = (moff + mi * P) // P
                for (toff, tsz) in _chunks(Tq, 512):
                    ps = f1_psum.tile([P, tsz], f32, tag="f1_ps")
                    for ct in range(CT):
                        nc.tensor.matmul(
                            ps, w1_cb[:, ct, mi * P:(mi + 1) * P],
                            xn2[ct][:, toff:toff + tsz],
                            start=(ct == 0), stop=(ct == CT - 1))
                    nc.scalar.activation(
                        hT[m][:, toff:toff + tsz], ps, gelu_func,
                        bias=pp["b1"][:, m:m + 1], scale=1.0)

    if cfg.get("stop_after") == 7:
        arena_cm.__exit__(None, None, None); const_cm.__exit__(None, None, None); return
    # ================= Phase 8: fc2 + residual 2 =================
    x2_pairs = [atile(f"v{i}", [P, 2, Tq], f32r, f"x2_{i}")
                for i in range(CT // 2)]
    x2 = [x2_pairs[ct // 2][:, ct % 2, :] for ct in range(CT)]
    with pool("w2f", bufs=2) as w2f_pool, \
         pool("w2h", bufs=2) as w2h_pool, \
         pool("f2ps", bufs=CT, space="PSUM") as f2_psum:
        # Stream w2 as contiguous row-tiles (cheap descriptors), once per
        # t-chunk, keeping CT open PSUM accumulators (one per c-out tile).
        for (toff, tsz) in _chunks(Tq, 512):
            pss = [f2_psum.tile([P, tsz], f32, tag="f2_ps",
                                name=f"f2ps{ct}") for ct in range(CT)]
            for ht in range(HT):
                w2_f32 = w2f_pool.tile([P, C], f32, tag="w2f32")
                nc.gpsimd.dma_start(
                    w2_f32, ins["w2"][ht * P:(ht + 1) * P, :])
                w2_f16 = w2h_pool.tile([P, C], f16, tag="w2f16")
                nc.vector.tensor_copy(w2_f16, w2_f32)
                for ct in range(CT):
                    nc.tensor.matmul(
                        pss[ct], w2_f16[:, ct * P:(ct + 1) * P],
                        hT[ht][:, toff:toff + tsz],
                        start=(ht == 0), stop=(ht == HT - 1))
            for ct in range(CT):
                nc.vector.scalar_tensor_tensor(
                    x2[ct][:, toff:toff + tsz], pss[ct], pp["b2"][:, ct:ct + 1],
                    x1[ct][:, toff:toff + tsz], OP.add, OP.add)

    if cfg.get("stop_after") == 8:
        arena_cm.__exit__(None, None, None); const_cm.__exit__(None, None, None); return
    # ================= Phase 9: out proj + softmax =================
    with pool("wout") as wpool, \
         pool("smax", bufs=3) as sm_pool, \
         pool("smsmall", bufs=6) as sms_pool, \
         pool("outps", bufs=4, space="PSUM") as out_psum:
        wout_sb = load_w_ctp(ins["w_out"], NCLS, wpool, "wout")
        n_chunks = _chunks(NCLS, 500)
        for tt in range(Tq // P):
            pss = []
            for (noff, nsz) in n_chunks:
                ps = out_psum.tile([P, nsz], f32, tag="out_ps")
                for ct in range(CT):
                    nc.tensor.matmul(
                        ps, x2[ct][:, tt * P:(tt + 1) * P],
                        wout_sb[:, ct, noff:noff + nsz],
                        start=(ct == 0), stop=False)
                nc.tensor.matmul(
                    ps, ones_r[0:1, :],
                    bout_sb[0:1, noff:noff + nsz],
                    start=False, stop=True)
                pss.append(ps)
            mx = sms_pool.tile([P, len(n_chunks)], f32, tag="sm_mx")
            for i, ps in enumerate(pss):
                nc.vector.reduce_max(mx[:, i:i + 1], ps, axis=AX.X)
            m = sms_pool.tile([P, 1], f32, tag="sm_m")
            nc.vector.reduce_max(m, mx, axis=AX.X)
            negm = sms_pool.tile([P, 1], f32, tag="sm_negm")
            nc.vector.tensor_scalar_mul(negm, m, -1.0)
            esb = sm_pool.tile([P, NCLS], f32, tag="sm_e")
            accs = sms_pool.tile([P, len(n_chunks)], f32, tag="sm_acc")
            for i, ((noff, nsz), ps) in enumerate(zip(n_chunks, pss)):
                nc.scalar.activation(
                    esb[:, noff:noff + nsz], ps, AF.Exp,
                    bias=negm, scale=1.0, accum_out=accs[:, i:i + 1])
            s = sms_pool.tile([P, 1], f32, tag="sm_s")
            nc.vector.reduce_sum(s, accs, axis=AX.X)
            rec = sms_pool.tile([P, 1], f32, tag="sm_rec")
            nc.vector.reciprocal(rec, s)
            nc.vector.tensor_scalar_mul(esb, esb, rec)
            nc.sync.dma_start(out_ap[tt * P:(tt + 1) * P, :], esb)

    arena_cm.__exit__(None, None, None)
    const_cm.__exit__(None, None, None)


# ======================= host entry =======================

_IN_NAMES = ["xs", "vals", "wq", "wk", "wv", "w_ap", "b_ap", "g1", "be1",
             "g2", "be2", "w1", "b1", "w2", "b2", "w_out", "b_out"]


def _build_nc(cfg):
    import concourse.bacc as bacc
    import concourse.mybir as mybir
    import concourse.tile as tile

    Tq, Tkv, C = cfg["Tq"], cfg["Tkv"], cfg["C"]
    HID, NCLS = cfg["HID"], cfg["NCLS"]
    shapes = dict(
        xs=[Tkv, C], vals=[Tkv, C], wq=[C, C], wk=[C, C], wv=[C, C],
        w_ap=[C, C], b_ap=[C], g1=[C], be1=[C], g2=[C], be2=[C],
        w1=[C, HID], b1=[HID], w2=[HID, C], b2=[C],
        w_out=[C, NCLS], b_out=[NCLS],
    )
    nc = bacc.Bacc("TRN2", target_bir_lowering=False, debug=False)
    ins = {k: nc.dram_tensor(k, shapes[k], mybir.dt.float32,
                             kind="ExternalInput").ap()
           for k in _IN_NAMES}
    out_ap = nc.dram_tensor("out", [Tq, NCLS], mybir.dt.float32,
                            kind="ExternalOutput").ap()
    with tile.TileContext(nc) as tc:
        emit_block(tc, out_ap, ins, cfg)
    nc.finalize()
    return nc


_NC_CACHE = {}


def kernel(**inputs) -> np.ndarray:
    from concourse.bass_utils import run_bass_kernel_spmd

    cfg = CFG_FULL
    B, N = cfg["B"], cfg["N"]
    Tq, NCLS = cfg["Tq"], cfg["NCLS"]
    n_cores = 8
    halves = N // Tq  # 2

    if "full" not in _NC_CACHE:
        _NC_CACHE["full"] = _build_nc(cfg)
    nc = _NC_CACHE["full"]

    x = np.ascontiguousarray(np.asarray(inputs["x"], dtype=np.float32))
    value = np.ascontiguousarray(np.asarray(inputs["value"], dtype=np.float32))
    shared = {k: np.ascontiguousarray(np.asarray(inputs[k], dtype=np.float32))
              for k in _IN_NAMES if k not in ("xs", "vals")}

    in_maps = []
    for core in range(n_cores):
        b, hf = core // halves, core % halves
        m = dict(shared)
        m["xs"] = np.ascontiguousarray(np.roll(x[b], -hf * Tq, axis=0))
        m["vals"] = np.ascontiguousarray(np.roll(value[b], -hf * Tq, axis=0))
        in_maps.append(m)

    res = run_bass_kernel_spmd(nc, in_maps, core_ids=list(range(n_cores)))
    out = np.empty((B, N, NCLS), dtype=np.float32)
    for core in range(n_cores):
        b, hf = core // halves, core % halves
        out[b, hf * Tq:(hf + 1) * Tq, :] = res.results[core]["out"]
    return out


# revision 25
# speedup vs baseline: 65.7289x; 1.0577x over previous
"""Trainium2 Bass kernel for a dense transformer block.

Sharding: 8-way SPMD, one (batch, half-sequence) shard of Tq=1024 query tokens
per core. Each core recomputes K/V for its whole batch (x/value rows are
host-rolled so the core's query tokens come first; softmax over keys is
permutation invariant). No collectives.

Layout: activations live transposed in SBUF as X^T [channel, token] so every
linear layer is matmul(lhsT=W[cin,cout], rhs=X^T) producing Y^T directly.
Dense GEMMs run as float32r (full-rate fp32 mode, free dim >= 256); attention
internals (Q/K/V, probs) are fp16 with fp32 PSUM accumulation. LN1 computes
stats in row-major layout (free-dim reduces) before transposing; LN2 computes
stats with ones-matmuls (cross-partition sums) giving partition-replicated
stats. Attention uses an S^T dataflow: S^T[k,q] = matmul(lhsT=K^T chunk,
rhs=Q^T) puts keys on partitions, so exp(S^T) (max-subtraction skipped —
scores are bounded) is directly the AV moving operand with no PE transposes
or PSUM->SBUF prob copies; a ones column appended to V makes PSUM row 64 the
softmax denominator, and 1/den is partition-broadcast with a K=1 ones-matmul
and folded into the O^T copy. Long-lived tensors share one SBUF pool with
explicit tag-slot reuse across phases (xn->OT->h, KT->x1, V->xq->xn2->x2).
"""

import sys

import numpy as np

if "/opt/trn_rl_repo" not in sys.path:
    sys.path.insert(0, "/opt/trn_rl_repo")

CFG_FULL = dict(
    Tq=1024, Tkv=2048, C=1024, H=16, D=64, HID=4096, NCLS=1000, EPS=1e-5,
    B=4, N=2048,
)


def _chunks(total, size):
    out = []
    s = 0
    while s < total:
        c = min(size, total - s)
        out.append((s, c))
        s += c
    return out


def emit_block(tc, out_ap, ins, cfg):
    """Emit the full transformer-block program for one core's shard."""
    import concourse.mybir as mybir
    from concourse.masks import make_identity

    nc = tc.nc
    f32 = mybir.dt.float32
    f16 = mybir.dt.float16
    f32r = mybir.dt.float32r
    AF = mybir.ActivationFunctionType
    OP = mybir.AluOpType
    AX = mybir.AxisListType

    Tq, Tkv, C, H, D = cfg["Tq"], cfg["Tkv"], cfg["C"], cfg["H"], cfg["D"]
    HID, NCLS, EPS = cfg["HID"], cfg["NCLS"], cfg["EPS"]
    P = 128
    CT = C // P
    KT = Tkv // P
    HT = HID // P
    VQ = max(1, KT // 4)      # V stored as 4 quarter tiles
    NVT = (KT + VQ - 1) // VQ
    HPK = min(HT, 4)          # h tiles packed 4 per slot
    NHT = (HT + HPK - 1) // HPK
    SCALE = C ** -0.5
    gelu_func = AF.Tanh if cfg.get("sim_gelu_tanh") else AF.Gelu

    assert H * D == C and D == 64 and C % P == 0 and Tkv % P == 0
    assert Tq % P == 0 and HID % P == 0 and H % 2 == 0 and CT % 2 == 0
    assert KT % VQ == 0 and HT % HPK == 0

    def r32(ap):
        return ap.bitcast(f32r)

    def pool(name, bufs=1, space="SBUF"):
        return tc.tile_pool(name=name, bufs=bufs, space=space)

    # ---------------- constants & params ----------------
    const_cm = pool("const")
    const_pool = const_cm.__enter__()

    ident32 = const_pool.tile([P, P], f32)
    make_identity(nc, ident32)
    ones128 = const_pool.tile([P, P], f32)
    nc.vector.memset(ones128, 1.0)
    ones_r = const_pool.tile([P, P], f32r)
    nc.vector.memset(ones_r.bitcast(f32), 1.0)
    eps_ap = const_pool.tile([P, 1], f32)
    nc.vector.memset(eps_ap, EPS)

    pp = {}
    with pool("ppps", bufs=2, space="PSUM") as psum_misc:
        def load_pp(vec_ap, n, key):
            nt = n // P
            ld = const_pool.tile([nt, P], f32, tag="pp_ld")
            nc.sync.dma_start(ld, vec_ap.rearrange("(a p) -> a p", p=P))
            ps = psum_misc.tile([P, nt], f32, tag="pp_ps")
            nc.tensor.matmul(ps, ld, ident32[:nt, :nt], is_transpose=True)
            dst = const_pool.tile([P, nt], f32, tag=f"pp_{key}")
            nc.vector.tensor_copy(dst, ps)
            pp[key] = dst

        for key in ["g1", "be1", "g2", "be2", "b_ap", "b2"]:
            load_pp(ins[key], C, key)
        load_pp(ins["b1"], HID, "b1")
    bout_sb = const_pool.tile([1, NCLS], f32r)
    nc.sync.dma_start(bout_sb, ins["b_out"][None, :].bitcast(f32r))

    # ---------------- long-lived arena ----------------
    arena_cm = pool("arena")
    arena = arena_cm.__enter__()

    def atile(slot, shape, dtype, name):
        return arena.tile(shape, dtype, tag=slot, name=name)

    # ---------------- helpers ----------------
    def emit_rows_to_T(rows_ap, T, dst_tiles, load_pool, tpsum):
        """DRAM [T, C] fp32 -> dst_tiles[ct][:, 0:T] = X^T tiles [128, T]."""
        ntt = T // P
        for tg in range(0, ntt, 4):
            gsz = min(4, ntt - tg)
            rows = []
            for j in range(gsz):
                r = load_pool.tile([P, C], f32, tag="rowload")
                nc.sync.dma_start(r, rows_ap[(tg + j) * P:(tg + j + 1) * P, :])
                rows.append(r)
            for ct in range(CT):
                ps = tpsum.tile([P, 4, P], f32, tag="tr_ps")
                for j in range(gsz):
                    nc.tensor.matmul(
                        ps[:, j, :], rows[j][:, ct * P:(ct + 1) * P], ident32,
                        is_transpose=True, start=(j == 0), stop=(j == gsz - 1),
                    )
                nc.vector.tensor_copy(
                    dst_tiles[ct][:, tg * P:(tg + gsz) * P],
                    ps[:, :gsz, :].rearrange("p g q -> p (g q)"),
                )

    def emit_layernorm_T(xT_tiles, T, g_pp, be_pp, dst_tiles, spool, stat_pool):
        """LayerNorm on transposed input (stats via ones-matmuls)."""
        for (toff, tsz) in _chunks(T, 512):
            s1 = spool.tile([P, tsz], f32, tag="ln_s1")
            s2 = spool.tile([P, tsz], f32, tag="ln_s2")
            for ct in range(CT):
                xc = xT_tiles[ct][:, toff:toff + tsz]
                nc.tensor.matmul(s1, ones128, xc,
                                 start=(ct == 0), stop=(ct == CT - 1))
                sq = stat_pool.tile([P, tsz], f32r, tag="ln_sq")
                nc.vector.tensor_tensor(sq, xc, xc, OP.mult)
                nc.tensor.matmul(s2, ones_r, sq,
                                 start=(ct == 0), stop=(ct == CT - 1))
            mu = stat_pool.tile([P, tsz], f32, tag="ln_mu")
            nc.vector.tensor_scalar_mul(mu, s1, 1.0 / C)
            m2 = stat_pool.tile([P, tsz], f32, tag="ln_m2")
            nc.vector.tensor_scalar_mul(m2, s2, 1.0 / C)
            musq = stat_pool.tile([P, tsz], f32, tag="ln_musq")
            nc.vector.tensor_tensor(musq, mu, mu, OP.mult)
            var = stat_pool.tile([P, tsz], f32, tag="ln_var")
            nc.vector.tensor_tensor(var, m2, musq, OP.subtract)
            A = stat_pool.tile([P, tsz], f32, tag="ln_A")
            emit_rsqrt(A, var, stat_pool, tsz)
            Bt = stat_pool.tile([P, tsz], f32, tag="ln_B")
            nc.vector.scalar_tensor_tensor(Bt, mu, -1.0, A, OP.mult, OP.mult)
            for ct in range(CT):
                xc = xT_tiles[ct][:, toff:toff + tsz]
                u = stat_pool.tile([P, tsz], f32, tag="ln_u")
                nc.vector.tensor_tensor(u, xc, A, OP.mult)
                nc.vector.tensor_tensor(u, u, Bt, OP.add)
                nc.vector.tensor_scalar(
                    dst_tiles[ct][:, toff:toff + tsz], u,
                    g_pp[:, ct:ct + 1], be_pp[:, ct:ct + 1], OP.mult, OP.add)

    def emit_rsqrt(dst, var, stat_pool, tsz):
        """dst = 1/sqrt(var+eps), with one Newton refinement."""
        std = stat_pool.tile([P, tsz], f32, tag="rs_std")
        nc.scalar.activation(std, var, AF.Sqrt, bias=eps_ap, scale=1.0)
        r0 = stat_pool.tile([P, tsz], f32, tag="rs_r0")
        nc.vector.reciprocal(r0, std)
        vpe = stat_pool.tile([P, tsz], f32, tag="rs_vpe")
        nc.vector.tensor_scalar_add(vpe, var, EPS)
        t0 = stat_pool.tile([P, tsz], f32, tag="rs_t0")
        nc.vector.tensor_tensor(t0, r0, r0, OP.mult)
        nc.vector.tensor_tensor(t0, t0, vpe, OP.mult)
        nc.vector.tensor_scalar(t0, t0, -0.5, 1.5, OP.mult, OP.add)
        nc.vector.tensor_tensor(dst, r0, t0, OP.mult)

    def load_w_ctp(w_ap_, m_total, wpool, tag):
        """[C, M] DRAM -> [128, CT, M] SBUF (row-tiled, fp32r)."""
        w_sb = wpool.tile([P, CT, m_total], f32r, tag=tag)
        nc.sync.dma_start(
            w_sb, w_ap_.rearrange("(ct p) m -> p ct m", p=P).bitcast(f32r))
        return w_sb

    # ====== Phase 1: rows of x -> per-token LN1 stats -> xn^T ======
    xn_tiles = [atile(f"a{ct}", [P, Tkv], f32r, f"xn{ct}") for ct in range(CT)]
    with pool("ld1", bufs=6) as load_pool, \
         pool("lnrow", bufs=2) as row_stat, \
         pool("trps1", bufs=3, space="PSUM") as tpsum:
        ntt = Tkv // P
        for tg in range(0, ntt, 4):
            gsz = min(4, ntt - tg)
            rows = []
            for j in range(gsz):
                r = load_pool.tile([P, C], f32, tag="rowload")
                nc.sync.dma_start(
                    r, ins["xs"][(tg + j) * P:(tg + j + 1) * P, :])
                s1 = row_stat.tile([P, 1], f32, tag="r_s1")
                nc.vector.reduce_sum(s1, r, axis=AX.X)
                sq = row_stat.tile([P, C], f32, tag="r_sq")
                s2 = row_stat.tile([P, 1], f32, tag="r_s2")
                nc.vector.scalar_tensor_tensor(
                    sq, r, 1.0, r, OP.bypass, OP.mult, accum_out=s2)
                mu = row_stat.tile([P, 1], f32, tag="r_mu")
                nc.vector.tensor_scalar_mul(mu, s1, 1.0 / C)
                m2 = row_stat.tile([P, 1], f32, tag="r_m2")
                nc.vector.tensor_scalar_mul(m2, s2, 1.0 / C)
                musq = row_stat.tile([P, 1], f32, tag="r_musq")
                nc.vector.tensor_tensor(musq, mu, mu, OP.mult)
                var = row_stat.tile([P, 1], f32, tag="r_var")
                nc.vector.tensor_tensor(var, m2, musq, OP.subtract)
                rstd = row_stat.tile([P, 1], f32, tag="r_rstd")
                emit_rsqrt(rstd, var, row_stat, 1)
                negmu = row_stat.tile([P, 1], f32, tag="r_negmu")
                nc.vector.tensor_scalar_mul(negmu, mu, -1.0)
                # rows <- (x - mu) * rstd   (token-wise, in place)
                nc.vector.tensor_scalar(r, r, negmu, rstd, OP.add, OP.mult)
                rows.append(r)
            for ct in range(CT):
                ps = tpsum.tile([P, 4, P], f32, tag="tr_ps")
                for j in range(gsz):
                    nc.tensor.matmul(
                        ps[:, j, :], rows[j][:, ct * P:(ct + 1) * P], ident32,
                        is_transpose=True, start=(j == 0), stop=(j == gsz - 1))
                # xn^T <- psum * g[c] + be[c]
                nc.vector.tensor_scalar(
                    xn_tiles[ct][:, tg * P:(tg + gsz) * P],
                    ps[:, :gsz, :].rearrange("p g q -> p (g q)"),
                    pp["g1"][:, ct:ct + 1], pp["be1"][:, ct:ct + 1],
                    OP.mult, OP.add)

    if cfg.get("stop_after") == 1:
        arena_cm.__exit__(None, None, None); const_cm.__exit__(None, None, None); return
    # ================= Phase 2: Q^T, K^T (fp16) =================
    QT = [atile(f"q{i}", [P, Tq], f16, f"QT{i}") for i in range(CT)]
    KTt = [atile(f"k{i}", [P, Tkv], f16, f"KT{i}") for i in range(CT)]
    with pool("wqk") as wpool, \
         pool("qkps", bufs=4, space="PSUM") as qk_psum:
        for (w_ap_, dst, T) in [(ins["wq"], QT, Tq), (ins["wk"], KTt, Tkv)]:
            w_sb = load_w_ctp(w_ap_, C, wpool, "wqk")
            for m in range(CT):
                for (toff, tsz) in _chunks(T, 512):
                    ps = qk_psum.tile([P, tsz], f32, tag="qk_ps")
                    for ct in range(CT):
                        nc.tensor.matmul(
                            ps, w_sb[:, ct, m * P:(m + 1) * P],
                            xn_tiles[ct][:, toff:toff + tsz],
                            start=(ct == 0), stop=(ct == CT - 1))
                    nc.vector.tensor_copy(dst[m][:, toff:toff + tsz], ps)

    if cfg.get("stop_after") == 2:
        arena_cm.__exit__(None, None, None); const_cm.__exit__(None, None, None); return
    # ========== Phase 3: V (fp16, [k, head, d+ones]) ==========
    # Column D of each head's 65-wide slot is 1.0 so the AV matmul's output
    # row 64 accumulates the softmax denominator for free.
    DA = D + 1
    Vq = [atile(f"v{i}", [P, VQ, H, DA], f16, f"V{i}") for i in range(NVT)]
    for vq in Vq:
        nc.vector.memset(vq, 1.0)
    with pool("wv") as wpool, \
         pool("vld", bufs=3) as vload, \
         pool("vt", bufs=2) as vt_pool, \
         pool("vtps", bufs=3, space="PSUM") as vt_psum, \
         pool("vps", bufs=3, space="PSUM") as v_psum:
        wv_sb = load_w_ctp(ins["wv"], C, wpool, "wv")
        for kt in range(KT):
            rows = vload.tile([P, C], f32, tag="vrow")
            nc.sync.dma_start(rows, ins["vals"][kt * P:(kt + 1) * P, :])
            vT_kt = vt_pool.tile([P, CT, P], f32r, tag="vTkt")
            for g0 in range(0, CT, 4):
                gsz = min(4, CT - g0)
                ps = vt_psum.tile([P, 4, P], f32, tag="vt_ps")
                for j in range(gsz):
                    nc.tensor.matmul(
                        ps[:, j, :], rows[:, (g0 + j) * P:(g0 + j + 1) * P],
                        ident32, is_transpose=True,
                        start=(j == 0), stop=(j == gsz - 1))
                nc.vector.tensor_copy(vT_kt[:, g0:g0 + gsz, :], ps[:, :gsz, :])
            for (noff, nsz) in _chunks(C, 512):
                vp = v_psum.tile([P, nsz], f32, tag="v_ps")
                for ct in range(CT):
                    nc.tensor.matmul(
                        vp, vT_kt[:, ct, :],
                        wv_sb[:, ct, noff:noff + nsz],
                        start=(ct == 0), stop=(ct == CT - 1))
                h0 = noff // D
                nc.vector.tensor_copy(
                    Vq[kt // VQ][:, kt % VQ, h0:h0 + nsz // D, 0:D],
                    vp.rearrange("p (h d) -> p h d", d=D))

    if cfg.get("stop_after") == 3:
        arena_cm.__exit__(None, None, None); const_cm.__exit__(None, None, None); return
    # ================= Phase 4: attention (S^T dataflow) =================
    # S^T[k, q] = matmul(lhsT=K^T chunk, rhs=Q^T) puts keys on partitions, so
    # exp(S^T) is directly the AV matmul's moving operand — no PE transposes
    # and no PSUM->SBUF prob copies. AV's lhsT is [V_h | ones] so PSUM row 64
    # is the softmax denominator; 1/den is broadcast across partitions with a
    # K=1 ones-matmul and folded into the O^T copy.
    OT = [atile(f"a{hp}", [D, 2, Tq], f32r, f"OT{hp}") for hp in range(H // 2)]
    with pool("es", bufs=2) as es_pool, \
         pool("attsmall", bufs=4) as small_pool, \
         pool("sps", bufs=2, space="PSUM") as s_psum, \
         pool("ops", bufs=2, space="PSUM") as o_psum, \
         pool("bcps", bufs=2, space="PSUM") as bc_psum:
        for h in range(H):
            hp, hf = h // 2, h % 2
            qT_h = QT[hp][hf * D:(hf + 1) * D, :]
            kT_h = KTt[hp][hf * D:(hf + 1) * D, :]
            for (qoff, qsz) in _chunks(Tq, 512):
                ES = es_pool.tile([P, KT, qsz], f16, tag="ES")
                for kt2 in range(0, KT, 2):
                    kn = min(2, KT - kt2)
                    sp = s_psum.tile([P, 2, qsz], f32, tag="s_ps")
                    one_bank = qsz * 4 * kn <= 2048
                    for j in range(kn):
                        nc.tensor.matmul(
                            sp[:, j, :],
                            kT_h[:, (kt2 + j) * P:(kt2 + j + 1) * P],
                            qT_h[:, qoff:qoff + qsz],
                            start=(j == 0 if one_bank else True),
                            stop=(j == kn - 1 if one_bank else True))
                    nc.scalar.activation(
                        ES[:, kt2:kt2 + kn, :], sp[:, :kn, :], AF.Exp,
                        bias=0.0, scale=SCALE)
                op = o_psum.tile([DA, qsz], f32, tag="o_ps")
                for kt in range(KT):
                    nc.tensor.matmul(
                        op, Vq[kt // VQ][:, kt % VQ, h, :], ES[:, kt, :],
                        start=(kt == 0), stop=(kt == KT - 1))
                rec = small_pool.tile([DA, qsz], f32, tag="rec")
                nc.vector.reciprocal(rec[D:DA, :], op[D:DA, :])
                recr = small_pool.tile([DA, qsz], f32r, tag="recr")
                nc.vector.tensor_copy(recr[D:DA, :], rec[D:DA, :])
                bc = bc_psum.tile([D, qsz], f32, tag="bc")
                nc.tensor.matmul(bc, ones_r[D:D + 1, 0:D],
                                 recr[D:DA, :], start=True, stop=True)
                bc_sb = small_pool.tile([D, qsz], f32, tag="bc_sb")
                nc.scalar.copy(bc_sb, bc)
                nc.vector.tensor_tensor(
                    OT[hp][:, hf, qoff:qoff + qsz], op[0:D, :], bc_sb, OP.mult)

    if cfg.get("stop_after") == 4:
        arena_cm.__exit__(None, None, None); const_cm.__exit__(None, None, None); return
    # ================= Phase 5: attn proj + residual 1 =================
    x1 = [atile(f"k{ct}", [P, Tq], f32, f"x1_{ct}") for ct in range(CT)]
    with pool("xqld", bufs=6) as load_pool, \
         pool("wap", bufs=2) as wap_pool, \
         pool("xqps", bufs=3, space="PSUM") as tpsum, \
         pool("apps", bufs=4, space="PSUM") as ap_psum:
        xq_pairs = [atile(f"v{i}", [P, 2, Tq], f32, f"xq{i}")
                    for i in range(CT // 2)]
        xqT = [xq_pairs[ct // 2][:, ct % 2, :] for ct in range(CT)]
        emit_rows_to_T(ins["xs"][:Tq, :], Tq, xqT, load_pool, tpsum)
        wap_r = ins["w_ap"].rearrange("(a p) m -> p a m", p=D)
        for ct in range(CT):
            wap_ct = wap_pool.tile([D, H, P], f32r, tag="wap_ct")
            nc.gpsimd.dma_start(wap_ct, wap_r[:, :, ct * P:(ct + 1) * P].bitcast(f32r))
            for (toff, tsz) in _chunks(Tq, 512):
                ps = ap_psum.tile([P, tsz], f32, tag="ap_ps")
                for ht in range(H):
                    nc.tensor.matmul(
                        ps, wap_ct[:, ht, :],
                        OT[ht // 2][:, ht % 2, toff:toff + tsz],
                        start=(ht == 0), stop=(ht == H - 1))
                nc.vector.scalar_tensor_tensor(
                    x1[ct][:, toff:toff + tsz], ps, pp["b_ap"][:, ct:ct + 1],
                    xqT[ct][:, toff:toff + tsz], OP.add, OP.add)

    if cfg.get("stop_after") == 5:
        arena_cm.__exit__(None, None, None); const_cm.__exit__(None, None, None); return
    # ============ Phase 6+7: LN2, fc1 + gelu -> h^T fp16 ============
    with pool("ln2ps", bufs=2, space="PSUM") as ln_spool, \
         pool("ln2stat", bufs=1) as ln_stat, \
         pool("w1", bufs=2) as w1_pool, \
         pool("f1ps", bufs=4, space="PSUM") as f1_psum:
        xn2_pairs = [atile(f"v{i}", [P, 2, Tq], f32r, f"xn2_{i}")
                     for i in range(CT // 2)]
        xn2 = [xn2_pairs[ct // 2][:, ct % 2, :] for ct in range(CT)]
        emit_layernorm_T(x1, Tq, pp["g2"], pp["be2"], xn2, ln_spool, ln_stat)
        hT_g = [atile(f"a{g}", [P, HPK, Tq], f16, f"hT{g}")
                for g in range(NHT)]
        hT = [hT_g[m // HPK][:, m % HPK, :] for m in range(HT)]
        w1r = ins["w1"].rearrange("(ct p) m -> p ct m", p=P)
        # stream w1 in column chunks of 4 m-tiles (fewer, fatter descriptors)
        W1CH = min(4 * P, HID)
        for (moff, msz) in _chunks(HID, W1CH):
            w1_cb = w1_pool.tile([P, CT, W1CH], f32r, tag="w1cb")
            nc.gpsimd.dma_start(
                w1_cb[:, :, :msz],
                w1r[:, :, moff:moff + msz].bitcast(f32r))
            for mi in range(msz // P):
                m = (moff + mi * P) // P
                for (toff, tsz) in _chunks(Tq, 512):
                    ps = f1_psum.tile([P, tsz], f32, tag="f1_ps")
                    for ct in range(CT):
                        nc.tensor.matmul(
                            ps, w1_cb[:, ct, mi * P:(mi + 1) * P],
                            xn2[ct][:, toff:toff + tsz],
                            start=(ct == 0), stop=(ct == CT - 1))
                    nc.scalar.activation(
                        hT[m][:, toff:toff + tsz], ps, gelu_func,
                        bias=pp["b1"][:, m:m + 1], scale=1.0)

    if cfg.get("stop_after") == 7:
        arena_cm.__exit__(None, None, None); const_cm.__exit__(None, None, None); return
    # ================= Phase 8: fc2 + residual 2 =================
    x2_pairs = [atile(f"v{i}", [P, 2, Tq], f32r, f"x2_{i}")
                for i in range(CT // 2)]
    x2 = [x2_pairs[ct // 2][:, ct % 2, :] for ct in range(CT)]
    with pool("w2f", bufs=2) as w2f_pool, \
         pool("w2h", bufs=2) as w2h_pool, \
         pool("f2ps", bufs=4, space="PSUM") as f2_psum:
        w2r = ins["w2"].rearrange("(ht p) c -> p ht c", p=P)
        for ct in range(CT):
            w2_f32 = w2f_pool.tile([P, HT, P], f32, tag="w2f32")
            nc.gpsimd.dma_start(w2_f32, w2r[:, :, ct * P:(ct + 1) * P])
            w2_f16 = w2h_pool.tile([P, HT, P], f16, tag="w2f16")
            nc.vector.tensor_copy(w2_f16, w2_f32)
            for (toff, tsz) in _chunks(Tq, 512):
                ps = f2_psum.tile([P, tsz], f32, tag="f2_ps")
                for ht in range(HT):
                    nc.tensor.matmul(
                        ps, w2_f16[:, ht, :], hT[ht][:, toff:toff + tsz],
                        start=(ht == 0), stop=(ht == HT - 1))
                nc.vector.scalar_tensor_tensor(
                    x2[ct][:, toff:toff + tsz], ps, pp["b2"][:, ct:ct + 1],
                    x1[ct][:, toff:toff + tsz], OP.add, OP.add)

    if cfg.get("stop_after") == 8:
        arena_cm.__exit__(None, None, None); const_cm.__exit__(None, None, None); return
    # ================= Phase 9: out proj + softmax =================
    with pool("wout") as wpool, \
         pool("smax", bufs=3) as sm_pool, \
         pool("smsmall", bufs=6) as sms_pool, \
         pool("outps", bufs=4, space="PSUM") as out_psum:
        wout_sb = load_w_ctp(ins["w_out"], NCLS, wpool, "wout")
        n_chunks = _chunks(NCLS, 500)
        for tt in range(Tq // P):
            pss = []
            for (noff, nsz) in n_chunks:
                ps = out_psum.tile([P, nsz], f32, tag="out_ps")
                for ct in range(CT):
                    nc.tensor.matmul(
                        ps, x2[ct][:, tt * P:(tt + 1) * P],
                        wout_sb[:, ct, noff:noff + nsz],
                        start=(ct == 0), stop=False)
                nc.tensor.matmul(
                    ps, ones_r[0:1, :],
                    bout_sb[0:1, noff:noff + nsz],
                    start=False, stop=True)
                pss.append(ps)
            mx = sms_pool.tile([P, len(n_chunks)], f32, tag="sm_mx")
            for i, ps in enumerate(pss):
                nc.vector.reduce_max(mx[:, i:i + 1], ps, axis=AX.X)
            m = sms_pool.tile([P, 1], f32, tag="sm_m")
            nc.vector.reduce_max(m, mx, axis=AX.X)
            negm = sms_pool.tile([P, 1], f32, tag="sm_negm")
            nc.vector.tensor_scalar_mul(negm, m, -1.0)
            esb = sm_pool.tile([P, NCLS], f32, tag="sm_e")
            accs = sms_pool.tile([P, len(n_chunks)], f32, tag="sm_acc")
            for i, ((noff, nsz), ps) in enumerate(zip(n_chunks, pss)):
                nc.scalar.activation(
                    esb[:, noff:noff + nsz], ps, AF.Exp,
                    bias=negm, scale=1.0, accum_out=accs[:, i:i + 1])
            s = sms_pool.tile([P, 1], f32, tag="sm_s")
            nc.vector.reduce_sum(s, accs, axis=AX.X)
            rec = sms_pool.tile([P, 1], f32, tag="sm_rec")
            nc.vector.reciprocal(rec, s)
            nc.vector.tensor_scalar_mul(esb, esb, rec)
            nc.sync.dma_start(out_ap[tt * P:(tt + 1) * P, :], esb)

    arena_cm.__exit__(None, None, None)
    const_cm.__exit__(None, None, None)


# ======================= host entry =======================

_IN_NAMES = ["xs", "vals", "wq", "wk", "wv", "w_ap", "b_ap", "g1", "be1",
             "g2", "be2", "w1", "b1", "w2", "b2", "w_out", "b_out"]


def _build_nc(cfg):
    import concourse.bacc as bacc
    import concourse.mybir as mybir
    import concourse.tile as tile

    Tq, Tkv, C = cfg["Tq"], cfg["Tkv"], cfg["C"]
    HID, NCLS = cfg["HID"], cfg["NCLS"]
    shapes = dict(
        xs=[Tkv, C], vals=[Tkv, C], wq=[C, C], wk=[C, C], wv=[C, C],
        w_ap=[C, C], b_ap=[C], g1=[C], be1=[C], g2=[C], be2=[C],
        w1=[C, HID], b1=[HID], w2=[HID, C], b2=[C],
        w_out=[C, NCLS], b_out=[NCLS],
    )
    nc = bacc.Bacc("TRN2", target_bir_lowering=False, debug=False)
    ins = {k: nc.dram_tensor(k, shapes[k], mybir.dt.float32,
                             kind="ExternalInput").ap()
           for k in _IN_NAMES}
    out_ap = nc.dram_tensor("out", [Tq, NCLS], mybir.dt.float32,
                            kind="ExternalOutput").ap()
    with tile.TileContext(nc) as tc:
        emit_block(tc, out_ap, ins, cfg)
    nc.finalize()
    return nc


_NC_CACHE = {}


def kernel(**inputs) -> np.ndarray:
    from concourse.bass_utils import run_bass_kernel_spmd

    cfg = CFG_FULL
    B, N = cfg["B"], cfg["N"]
    Tq, NCLS = cfg["Tq"], cfg["NCLS"]
    n_cores = 8
    halves = N // Tq  # 2

    if "full" not in _NC_CACHE:
        _NC_CACHE["full"] = _build_nc(cfg)
    nc = _NC_CACHE["full"]

    x = np.ascontiguousarray(np.asarray(inputs["x"], dtype=np.float32))
    value = np.ascontiguousarray(np.asarray(inputs["value"], dtype=np.float32))
    shared = {k: np.ascontiguousarray(np.asarray(inputs[k], dtype=np.float32))
              for k in _IN_NAMES if k not in ("xs", "vals")}

    in_maps = []
    for core in range(n_cores):
        b, hf = core // halves, core % halves
        m = dict(shared)
        m["xs"] = np.ascontiguousarray(np.roll(x[b], -hf * Tq, axis=0))
        m["vals"] = np.ascontiguousarray(np.roll(value[b], -hf * Tq, axis=0))
        in_maps.append(m)

    res = run_bass_kernel_spmd(nc, in_maps, core_ids=list(range(n_cores)))
    out = np.empty((B, N, NCLS), dtype=np.float32)
    for core in range(n_cores):
        b, hf = core // halves, core % halves
        out[b, hf * Tq:(hf + 1) * Tq, :] = res.results[core]["out"]
    return out


# revision 26
# speedup vs baseline: 72.9909x; 1.1105x over previous
"""Trainium2 Bass kernel for a dense transformer block.

Sharding: 8-way SPMD, one (batch, half-sequence) shard of Tq=1024 query tokens
per core. Each core recomputes K/V for its whole batch (x/value rows are
host-rolled so the core's query tokens come first; softmax over keys is
permutation invariant). No collectives.

Layout: activations live transposed in SBUF as X^T [channel, token] so every
linear layer is matmul(lhsT=W[cin,cout], rhs=X^T) producing Y^T directly.
Dense GEMMs run as float32r (full-rate fp32 mode, free dim >= 256); attention
internals (Q/K/V, probs) are fp16 with fp32 PSUM accumulation. LN1 computes
stats in row-major layout (free-dim reduces) before transposing; LN2 computes
stats with ones-matmuls (cross-partition sums) giving partition-replicated
stats. Attention uses an S^T dataflow: S^T[k,q] = matmul(lhsT=K^T chunk,
rhs=Q^T) puts keys on partitions, so exp(S^T) (max-subtraction skipped —
scores are bounded) is directly the AV moving operand with no PE transposes
or PSUM->SBUF prob copies; a ones column appended to V makes PSUM row 64 the
softmax denominator, and 1/den is partition-broadcast with a K=1 ones-matmul
and folded into the O^T copy. Long-lived tensors share one SBUF pool with
explicit tag-slot reuse across phases (xn->OT->h, KT->x1, V->xq->xn2->x2).
"""

import sys

import numpy as np

if "/opt/trn_rl_repo" not in sys.path:
    sys.path.insert(0, "/opt/trn_rl_repo")

CFG_FULL = dict(
    Tq=1024, Tkv=2048, C=1024, H=16, D=64, HID=4096, NCLS=1000, EPS=1e-5,
    B=4, N=2048,
)


def _chunks(total, size):
    out = []
    s = 0
    while s < total:
        c = min(size, total - s)
        out.append((s, c))
        s += c
    return out


def emit_block(tc, out_ap, ins, cfg):
    """Emit the full transformer-block program for one core's shard."""
    import concourse.mybir as mybir
    from concourse.masks import make_identity

    nc = tc.nc
    f32 = mybir.dt.float32
    f16 = mybir.dt.float16
    f32r = mybir.dt.float32r
    AF = mybir.ActivationFunctionType
    OP = mybir.AluOpType
    AX = mybir.AxisListType

    Tq, Tkv, C, H, D = cfg["Tq"], cfg["Tkv"], cfg["C"], cfg["H"], cfg["D"]
    HID, NCLS, EPS = cfg["HID"], cfg["NCLS"], cfg["EPS"]
    P = 128
    CT = C // P
    KT = Tkv // P
    HT = HID // P
    VQ = max(1, KT // 4)      # V stored as 4 quarter tiles
    NVT = (KT + VQ - 1) // VQ
    HPK = min(HT, 4)          # h tiles packed 4 per slot
    NHT = (HT + HPK - 1) // HPK
    SCALE = C ** -0.5
    gelu_func = AF.Tanh if cfg.get("sim_gelu_tanh") else AF.Gelu

    assert H * D == C and D == 64 and C % P == 0 and Tkv % P == 0
    assert Tq % P == 0 and HID % P == 0 and H % 2 == 0 and CT % 2 == 0
    assert KT % VQ == 0 and HT % HPK == 0

    def r32(ap):
        return ap.bitcast(f32r)

    def pool(name, bufs=1, space="SBUF"):
        return tc.tile_pool(name=name, bufs=bufs, space=space)

    # ---------------- constants & params ----------------
    const_cm = pool("const")
    const_pool = const_cm.__enter__()

    ident32 = const_pool.tile([P, P], f32)
    make_identity(nc, ident32)
    ones128 = const_pool.tile([P, P], f32)
    nc.vector.memset(ones128, 1.0)
    ones_r = const_pool.tile([P, P], f32r)
    nc.vector.memset(ones_r.bitcast(f32), 1.0)
    eps_ap = const_pool.tile([P, 1], f32)
    nc.vector.memset(eps_ap, EPS)

    pp = {}
    with pool("ppps", bufs=2, space="PSUM") as psum_misc:
        def load_pp(vec_ap, n, key):
            nt = n // P
            ld = const_pool.tile([nt, P], f32, tag="pp_ld")
            nc.sync.dma_start(ld, vec_ap.rearrange("(a p) -> a p", p=P))
            ps = psum_misc.tile([P, nt], f32, tag="pp_ps")
            nc.tensor.matmul(ps, ld, ident32[:nt, :nt], is_transpose=True)
            dst = const_pool.tile([P, nt], f32, tag=f"pp_{key}")
            nc.vector.tensor_copy(dst, ps)
            pp[key] = dst

        for key in ["g1", "be1", "g2", "be2", "b_ap", "b2"]:
            load_pp(ins[key], C, key)
        load_pp(ins["b1"], HID, "b1")
    bout_sb = const_pool.tile([1, NCLS], f32r)
    nc.sync.dma_start(bout_sb, ins["b_out"][None, :].bitcast(f32r))

    # ---------------- long-lived arena ----------------
    arena_cm = pool("arena")
    arena = arena_cm.__enter__()

    def atile(slot, shape, dtype, name):
        return arena.tile(shape, dtype, tag=slot, name=name)

    # ---------------- helpers ----------------
    def emit_rows_to_T(rows_ap, T, dst_tiles, load_pool, tpsum):
        """DRAM [T, C] fp32 -> dst_tiles[ct][:, 0:T] = X^T tiles [128, T]."""
        ntt = T // P
        for tg in range(0, ntt, 4):
            gsz = min(4, ntt - tg)
            rows = []
            for j in range(gsz):
                r = load_pool.tile([P, C], f32, tag="rowload")
                nc.sync.dma_start(r, rows_ap[(tg + j) * P:(tg + j + 1) * P, :])
                rows.append(r)
            for ct in range(CT):
                ps = tpsum.tile([P, 4, P], f32, tag="tr_ps")
                for j in range(gsz):
                    nc.tensor.matmul(
                        ps[:, j, :], rows[j][:, ct * P:(ct + 1) * P], ident32,
                        is_transpose=True, start=(j == 0), stop=(j == gsz - 1),
                    )
                nc.vector.tensor_copy(
                    dst_tiles[ct][:, tg * P:(tg + gsz) * P],
                    ps[:, :gsz, :].rearrange("p g q -> p (g q)"),
                )

    def emit_layernorm_T(xT_tiles, T, g_pp, be_pp, dst_tiles, spool, stat_pool):
        """LayerNorm on transposed input (stats via ones-matmuls)."""
        for (toff, tsz) in _chunks(T, 512):
            s1 = spool.tile([P, tsz], f32, tag="ln_s1")
            s2 = spool.tile([P, tsz], f32, tag="ln_s2")
            for ct in range(CT):
                xc = xT_tiles[ct][:, toff:toff + tsz]
                nc.tensor.matmul(s1, ones128, xc,
                                 start=(ct == 0), stop=(ct == CT - 1))
                sq = stat_pool.tile([P, tsz], f32r, tag="ln_sq")
                nc.vector.tensor_tensor(sq, xc, xc, OP.mult)
                nc.tensor.matmul(s2, ones_r, sq,
                                 start=(ct == 0), stop=(ct == CT - 1))
            mu = stat_pool.tile([P, tsz], f32, tag="ln_mu")
            nc.vector.tensor_scalar_mul(mu, s1, 1.0 / C)
            m2 = stat_pool.tile([P, tsz], f32, tag="ln_m2")
            nc.vector.tensor_scalar_mul(m2, s2, 1.0 / C)
            musq = stat_pool.tile([P, tsz], f32, tag="ln_musq")
            nc.vector.tensor_tensor(musq, mu, mu, OP.mult)
            var = stat_pool.tile([P, tsz], f32, tag="ln_var")
            nc.vector.tensor_tensor(var, m2, musq, OP.subtract)
            A = stat_pool.tile([P, tsz], f32, tag="ln_A")
            emit_rsqrt(A, var, stat_pool, tsz)
            Bt = stat_pool.tile([P, tsz], f32, tag="ln_B")
            nc.vector.scalar_tensor_tensor(Bt, mu, -1.0, A, OP.mult, OP.mult)
            for ct in range(CT):
                xc = xT_tiles[ct][:, toff:toff + tsz]
                u = stat_pool.tile([P, tsz], f32, tag="ln_u")
                nc.vector.tensor_tensor(u, xc, A, OP.mult)
                nc.vector.tensor_tensor(u, u, Bt, OP.add)
                nc.vector.tensor_scalar(
                    dst_tiles[ct][:, toff:toff + tsz], u,
                    g_pp[:, ct:ct + 1], be_pp[:, ct:ct + 1], OP.mult, OP.add)

    def emit_rsqrt(dst, var, stat_pool, tsz):
        """dst = 1/sqrt(var+eps), with one Newton refinement."""
        std = stat_pool.tile([P, tsz], f32, tag="rs_std")
        nc.scalar.activation(std, var, AF.Sqrt, bias=eps_ap, scale=1.0)
        r0 = stat_pool.tile([P, tsz], f32, tag="rs_r0")
        nc.vector.reciprocal(r0, std)
        vpe = stat_pool.tile([P, tsz], f32, tag="rs_vpe")
        nc.vector.tensor_scalar_add(vpe, var, EPS)
        t0 = stat_pool.tile([P, tsz], f32, tag="rs_t0")
        nc.vector.tensor_tensor(t0, r0, r0, OP.mult)
        nc.vector.tensor_tensor(t0, t0, vpe, OP.mult)
        nc.vector.tensor_scalar(t0, t0, -0.5, 1.5, OP.mult, OP.add)
        nc.vector.tensor_tensor(dst, r0, t0, OP.mult)

    def load_w_ctp(w_ap_, m_total, wpool, tag):
        """[C, M] DRAM -> [128, CT, M] SBUF (row-tiled, fp32r)."""
        w_sb = wpool.tile([P, CT, m_total], f32r, tag=tag)
        nc.sync.dma_start(
            w_sb, w_ap_.rearrange("(ct p) m -> p ct m", p=P).bitcast(f32r))
        return w_sb

    # ====== Phase 1: rows of x -> per-token LN1 stats -> xn^T ======
    xn_tiles = [atile(f"a{ct}", [P, Tkv], f32r, f"xn{ct}") for ct in range(CT)]
    wqk_cm = pool("wqk")
    wqk_pool = wqk_cm.__enter__()
    wq_sb = load_w_ctp(ins["wq"], C, wqk_pool, "wqk")  # overlaps LN1
    with pool("ld1", bufs=4) as load_pool, \
         pool("lnrow", bufs=2) as row_stat, \
         pool("trps1", bufs=3, space="PSUM") as tpsum:
        ntt = Tkv // P
        for tg in range(0, ntt, 4):
            gsz = min(4, ntt - tg)
            rows = []
            for j in range(gsz):
                r = load_pool.tile([P, C], f32, tag="rowload")
                nc.sync.dma_start(
                    r, ins["xs"][(tg + j) * P:(tg + j + 1) * P, :])
                s1 = row_stat.tile([P, 1], f32, tag="r_s1")
                nc.vector.reduce_sum(s1, r, axis=AX.X)
                sq = row_stat.tile([P, C], f32, tag="r_sq")
                s2 = row_stat.tile([P, 1], f32, tag="r_s2")
                nc.vector.scalar_tensor_tensor(
                    sq, r, 1.0, r, OP.bypass, OP.mult, accum_out=s2)
                mu = row_stat.tile([P, 1], f32, tag="r_mu")
                nc.vector.tensor_scalar_mul(mu, s1, 1.0 / C)
                m2 = row_stat.tile([P, 1], f32, tag="r_m2")
                nc.vector.tensor_scalar_mul(m2, s2, 1.0 / C)
                musq = row_stat.tile([P, 1], f32, tag="r_musq")
                nc.vector.tensor_tensor(musq, mu, mu, OP.mult)
                var = row_stat.tile([P, 1], f32, tag="r_var")
                nc.vector.tensor_tensor(var, m2, musq, OP.subtract)
                rstd = row_stat.tile([P, 1], f32, tag="r_rstd")
                emit_rsqrt(rstd, var, row_stat, 1)
                negmu = row_stat.tile([P, 1], f32, tag="r_negmu")
                nc.vector.tensor_scalar_mul(negmu, mu, -1.0)
                # rows <- (x - mu) * rstd   (token-wise, in place)
                nc.vector.tensor_scalar(r, r, negmu, rstd, OP.add, OP.mult)
                rows.append(r)
            for ct in range(CT):
                ps = tpsum.tile([P, 4, P], f32, tag="tr_ps")
                for j in range(gsz):
                    nc.tensor.matmul(
                        ps[:, j, :], rows[j][:, ct * P:(ct + 1) * P], ident32,
                        is_transpose=True, start=(j == 0), stop=(j == gsz - 1))
                # xn^T <- psum * g[c] + be[c]
                nc.vector.tensor_scalar(
                    xn_tiles[ct][:, tg * P:(tg + gsz) * P],
                    ps[:, :gsz, :].rearrange("p g q -> p (g q)"),
                    pp["g1"][:, ct:ct + 1], pp["be1"][:, ct:ct + 1],
                    OP.mult, OP.add)

    if cfg.get("stop_after") == 1:
        arena_cm.__exit__(None, None, None); const_cm.__exit__(None, None, None); return
    # ================= Phase 2: Q^T, K^T (fp16) =================
    QT = [atile(f"q{i}", [P, Tq], f16, f"QT{i}") for i in range(CT)]
    KTt = [atile(f"k{i}", [P, Tkv], f16, f"KT{i}") for i in range(CT)]
    with pool("qkps", bufs=4, space="PSUM") as qk_psum:
        for (w_sb_pre, w_ap_, dst, T) in [(wq_sb, None, QT, Tq),
                                          (None, ins["wk"], KTt, Tkv)]:
            w_sb = (w_sb_pre if w_sb_pre is not None
                    else load_w_ctp(w_ap_, C, wqk_pool, "wqk"))
            for m in range(CT):
                for (toff, tsz) in _chunks(T, 512):
                    ps = qk_psum.tile([P, tsz], f32, tag="qk_ps")
                    for ct in range(CT):
                        nc.tensor.matmul(
                            ps, w_sb[:, ct, m * P:(m + 1) * P],
                            xn_tiles[ct][:, toff:toff + tsz],
                            start=(ct == 0), stop=(ct == CT - 1))
                    nc.vector.tensor_copy(dst[m][:, toff:toff + tsz], ps)
    wqk_cm.__exit__(None, None, None)

    if cfg.get("stop_after") == 2:
        arena_cm.__exit__(None, None, None); const_cm.__exit__(None, None, None); return
    # ========== Phase 3: V (fp16, [k, head, d+ones]) ==========
    # Column D of each head's 65-wide slot is 1.0 so the AV matmul's output
    # row 64 accumulates the softmax denominator for free.
    DA = D + 1
    Vq = [atile(f"v{i}", [P, VQ, H, DA], f16, f"V{i}") for i in range(NVT)]
    for vq in Vq:
        nc.vector.memset(vq, 1.0)
    with pool("wv") as wpool, \
         pool("vld", bufs=3) as vload, \
         pool("vt", bufs=2) as vt_pool, \
         pool("vtps", bufs=3, space="PSUM") as vt_psum, \
         pool("vps", bufs=3, space="PSUM") as v_psum:
        wv_sb = load_w_ctp(ins["wv"], C, wpool, "wv")
        for kt in range(KT):
            rows = vload.tile([P, C], f32, tag="vrow")
            nc.sync.dma_start(rows, ins["vals"][kt * P:(kt + 1) * P, :])
            vT_kt = vt_pool.tile([P, CT, P], f32r, tag="vTkt")
            for g0 in range(0, CT, 4):
                gsz = min(4, CT - g0)
                ps = vt_psum.tile([P, 4, P], f32, tag="vt_ps")
                for j in range(gsz):
                    nc.tensor.matmul(
                        ps[:, j, :], rows[:, (g0 + j) * P:(g0 + j + 1) * P],
                        ident32, is_transpose=True,
                        start=(j == 0), stop=(j == gsz - 1))
                nc.vector.tensor_copy(vT_kt[:, g0:g0 + gsz, :], ps[:, :gsz, :])
            for (noff, nsz) in _chunks(C, 512):
                vp = v_psum.tile([P, nsz], f32, tag="v_ps")
                for ct in range(CT):
                    nc.tensor.matmul(
                        vp, vT_kt[:, ct, :],
                        wv_sb[:, ct, noff:noff + nsz],
                        start=(ct == 0), stop=(ct == CT - 1))
                h0 = noff // D
                nc.vector.tensor_copy(
                    Vq[kt // VQ][:, kt % VQ, h0:h0 + nsz // D, 0:D],
                    vp.rearrange("p (h d) -> p h d", d=D))

    if cfg.get("stop_after") == 3:
        arena_cm.__exit__(None, None, None); const_cm.__exit__(None, None, None); return
    # ================= Phase 4: attention (S^T dataflow) =================
    # S^T[k, q] = matmul(lhsT=K^T chunk, rhs=Q^T) puts keys on partitions, so
    # exp(S^T) is directly the AV matmul's moving operand — no PE transposes
    # and no PSUM->SBUF prob copies. AV's lhsT is [V_h | ones] so PSUM row 64
    # is the softmax denominator; 1/den is broadcast across partitions with a
    # K=1 ones-matmul and folded into the O^T copy.
    OT = [atile(f"a{hp}", [D, 2, Tq], f32r, f"OT{hp}") for hp in range(H // 2)]
    with pool("es", bufs=2) as es_pool, \
         pool("attsmall", bufs=4) as small_pool, \
         pool("sps", bufs=2, space="PSUM") as s_psum, \
         pool("ops", bufs=2, space="PSUM") as o_psum, \
         pool("bcps", bufs=2, space="PSUM") as bc_psum:
        for h in range(H):
            hp, hf = h // 2, h % 2
            qT_h = QT[hp][hf * D:(hf + 1) * D, :]
            kT_h = KTt[hp][hf * D:(hf + 1) * D, :]
            for (qoff, qsz) in _chunks(Tq, 512):
                ES = es_pool.tile([P, KT, qsz], f16, tag="ES")
                for kt2 in range(0, KT, 2):
                    kn = min(2, KT - kt2)
                    sp = s_psum.tile([P, 2, qsz], f32, tag="s_ps")
                    one_bank = qsz * 4 * kn <= 2048
                    for j in range(kn):
                        nc.tensor.matmul(
                            sp[:, j, :],
                            kT_h[:, (kt2 + j) * P:(kt2 + j + 1) * P],
                            qT_h[:, qoff:qoff + qsz],
                            start=(j == 0 if one_bank else True),
                            stop=(j == kn - 1 if one_bank else True))
                    nc.scalar.activation(
                        ES[:, kt2:kt2 + kn, :], sp[:, :kn, :], AF.Exp,
                        bias=0.0, scale=SCALE)
                op = o_psum.tile([DA, qsz], f32, tag="o_ps")
                for kt in range(KT):
                    nc.tensor.matmul(
                        op, Vq[kt // VQ][:, kt % VQ, h, :], ES[:, kt, :],
                        start=(kt == 0), stop=(kt == KT - 1))
                rec = small_pool.tile([DA, qsz], f32, tag="rec")
                nc.vector.reciprocal(rec[D:DA, :], op[D:DA, :])
                recr = small_pool.tile([DA, qsz], f32r, tag="recr")
                nc.vector.tensor_copy(recr[D:DA, :], rec[D:DA, :])
                bc = bc_psum.tile([D, qsz], f32, tag="bc")
                nc.tensor.matmul(bc, ones_r[D:D + 1, 0:D],
                                 recr[D:DA, :], start=True, stop=True)
                bc_sb = small_pool.tile([D, qsz], f32, tag="bc_sb")
                nc.scalar.copy(bc_sb, bc)
                nc.vector.tensor_tensor(
                    OT[hp][:, hf, qoff:qoff + qsz], op[0:D, :], bc_sb, OP.mult)

    if cfg.get("stop_after") == 4:
        arena_cm.__exit__(None, None, None); const_cm.__exit__(None, None, None); return
    # ================= Phase 5: attn proj + residual 1 =================
    x1 = [atile(f"k{ct}", [P, Tq], f32, f"x1_{ct}") for ct in range(CT)]
    with pool("xqld", bufs=6) as load_pool, \
         pool("wap", bufs=2) as wap_pool, \
         pool("xqps", bufs=3, space="PSUM") as tpsum, \
         pool("apps", bufs=4, space="PSUM") as ap_psum:
        xq_pairs = [atile(f"v{i}", [P, 2, Tq], f32, f"xq{i}")
                    for i in range(CT // 2)]
        xqT = [xq_pairs[ct // 2][:, ct % 2, :] for ct in range(CT)]
        emit_rows_to_T(ins["xs"][:Tq, :], Tq, xqT, load_pool, tpsum)
        wap_r = ins["w_ap"].rearrange("(a p) m -> p a m", p=D)
        for ct in range(CT):
            wap_ct = wap_pool.tile([D, H, P], f32r, tag="wap_ct")
            nc.gpsimd.dma_start(wap_ct, wap_r[:, :, ct * P:(ct + 1) * P].bitcast(f32r))
            for (toff, tsz) in _chunks(Tq, 512):
                ps = ap_psum.tile([P, tsz], f32, tag="ap_ps")
                for ht in range(H):
                    nc.tensor.matmul(
                        ps, wap_ct[:, ht, :],
                        OT[ht // 2][:, ht % 2, toff:toff + tsz],
                        start=(ht == 0), stop=(ht == H - 1))
                nc.vector.scalar_tensor_tensor(
                    x1[ct][:, toff:toff + tsz], ps, pp["b_ap"][:, ct:ct + 1],
                    xqT[ct][:, toff:toff + tsz], OP.add, OP.add)

    if cfg.get("stop_after") == 5:
        arena_cm.__exit__(None, None, None); const_cm.__exit__(None, None, None); return
    # ============ Phase 6+7: LN2, fc1 + gelu -> h^T fp16 ============
    with pool("ln2ps", bufs=2, space="PSUM") as ln_spool, \
         pool("ln2stat", bufs=1) as ln_stat, \
         pool("w1", bufs=2) as w1_pool, \
         pool("f1ps", bufs=4, space="PSUM") as f1_psum:
        xn2_pairs = [atile(f"v{i}", [P, 2, Tq], f32r, f"xn2_{i}")
                     for i in range(CT // 2)]
        xn2 = [xn2_pairs[ct // 2][:, ct % 2, :] for ct in range(CT)]
        emit_layernorm_T(x1, Tq, pp["g2"], pp["be2"], xn2, ln_spool, ln_stat)
        hT_g = [atile(f"a{g}", [P, HPK, Tq], f16, f"hT{g}")
                for g in range(NHT)]
        hT = [hT_g[m // HPK][:, m % HPK, :] for m in range(HT)]
        w1r = ins["w1"].rearrange("(ct p) m -> p ct m", p=P)
        # stream w1 in column chunks of 4 m-tiles (fewer, fatter descriptors)
        W1CH = min(4 * P, HID)
        for (moff, msz) in _chunks(HID, W1CH):
            w1_cb = w1_pool.tile([P, CT, W1CH], f32r, tag="w1cb")
            nc.gpsimd.dma_start(
                w1_cb[:, :, :msz],
                w1r[:, :, moff:moff + msz].bitcast(f32r))
            for mi in range(msz // P):
                m = (moff + mi * P) // P
                for (toff, tsz) in _chunks(Tq, 512):
                    ps = f1_psum.tile([P, tsz], f32, tag="f1_ps")
                    for ct in range(CT):
                        nc.tensor.matmul(
                            ps, w1_cb[:, ct, mi * P:(mi + 1) * P],
                            xn2[ct][:, toff:toff + tsz],
                            start=(ct == 0), stop=(ct == CT - 1))
                    nc.scalar.activation(
                        hT[m][:, toff:toff + tsz], ps, gelu_func,
                        bias=pp["b1"][:, m:m + 1], scale=1.0)

    if cfg.get("stop_after") == 7:
        arena_cm.__exit__(None, None, None); const_cm.__exit__(None, None, None); return
    # ================= Phase 8: fc2 + residual 2 =================
    x2_pairs = [atile(f"v{i}", [P, 2, Tq], f32r, f"x2_{i}")
                for i in range(CT // 2)]
    x2 = [x2_pairs[ct // 2][:, ct % 2, :] for ct in range(CT)]
    with pool("w2f", bufs=2) as w2f_pool, \
         pool("w2h", bufs=2) as w2h_pool, \
         pool("f2ps", bufs=4, space="PSUM") as f2_psum:
        w2r = ins["w2"].rearrange("(ht p) c -> p ht c", p=P)
        for ct in range(CT):
            w2_f32 = w2f_pool.tile([P, HT, P], f32, tag="w2f32")
            nc.gpsimd.dma_start(w2_f32, w2r[:, :, ct * P:(ct + 1) * P])
            w2_f16 = w2h_pool.tile([P, HT, P], f16, tag="w2f16")
            nc.vector.tensor_copy(w2_f16, w2_f32)
            for (toff, tsz) in _chunks(Tq, 512):
                ps = f2_psum.tile([P, tsz], f32, tag="f2_ps")
                for ht in range(HT):
                    nc.tensor.matmul(
                        ps, w2_f16[:, ht, :], hT[ht][:, toff:toff + tsz],
                        start=(ht == 0), stop=(ht == HT - 1))
                nc.vector.scalar_tensor_tensor(
                    x2[ct][:, toff:toff + tsz], ps, pp["b2"][:, ct:ct + 1],
                    x1[ct][:, toff:toff + tsz], OP.add, OP.add)

    if cfg.get("stop_after") == 8:
        arena_cm.__exit__(None, None, None); const_cm.__exit__(None, None, None); return
    # ================= Phase 9: out proj + softmax =================
    with pool("wout") as wpool, \
         pool("smax", bufs=3) as sm_pool, \
         pool("smsmall", bufs=6) as sms_pool, \
         pool("outps", bufs=4, space="PSUM") as out_psum:
        wout_sb = load_w_ctp(ins["w_out"], NCLS, wpool, "wout")
        n_chunks = _chunks(NCLS, 500)
        for tt in range(Tq // P):
            pss = []
            for (noff, nsz) in n_chunks:
                ps = out_psum.tile([P, nsz], f32, tag="out_ps")
                for ct in range(CT):
                    nc.tensor.matmul(
                        ps, x2[ct][:, tt * P:(tt + 1) * P],
                        wout_sb[:, ct, noff:noff + nsz],
                        start=(ct == 0), stop=False)
                nc.tensor.matmul(
                    ps, ones_r[0:1, :],
                    bout_sb[0:1, noff:noff + nsz],
                    start=False, stop=True)
                pss.append(ps)
            mx = sms_pool.tile([P, len(n_chunks)], f32, tag="sm_mx")
            for i, ps in enumerate(pss):
                nc.vector.reduce_max(mx[:, i:i + 1], ps, axis=AX.X)
            m = sms_pool.tile([P, 1], f32, tag="sm_m")
            nc.vector.reduce_max(m, mx, axis=AX.X)
            negm = sms_pool.tile([P, 1], f32, tag="sm_negm")
            nc.vector.tensor_scalar_mul(negm, m, -1.0)
            esb = sm_pool.tile([P, NCLS], f32, tag="sm_e")
            accs = sms_pool.tile([P, len(n_chunks)], f32, tag="sm_acc")
            for i, ((noff, nsz), ps) in enumerate(zip(n_chunks, pss)):
                nc.scalar.activation(
                    esb[:, noff:noff + nsz], ps, AF.Exp,
                    bias=negm, scale=1.0, accum_out=accs[:, i:i + 1])
            s = sms_pool.tile([P, 1], f32, tag="sm_s")
            nc.vector.reduce_sum(s, accs, axis=AX.X)
            rec = sms_pool.tile([P, 1], f32, tag="sm_rec")
            nc.vector.reciprocal(rec, s)
            nc.vector.tensor_scalar_mul(esb, esb, rec)
            nc.sync.dma_start(out_ap[tt * P:(tt + 1) * P, :], esb)

    arena_cm.__exit__(None, None, None)
    const_cm.__exit__(None, None, None)


# ======================= host entry =======================

_IN_NAMES = ["xs", "vals", "wq", "wk", "wv", "w_ap", "b_ap", "g1", "be1",
             "g2", "be2", "w1", "b1", "w2", "b2", "w_out", "b_out"]


def _build_nc(cfg):
    import concourse.bacc as bacc
    import concourse.mybir as mybir
    import concourse.tile as tile

    Tq, Tkv, C = cfg["Tq"], cfg["Tkv"], cfg["C"]
    HID, NCLS = cfg["HID"], cfg["NCLS"]
    shapes = dict(
        xs=[Tkv, C], vals=[Tkv, C], wq=[C, C], wk=[C, C], wv=[C, C],
        w_ap=[C, C], b_ap=[C], g1=[C], be1=[C], g2=[C], be2=[C],
        w1=[C, HID], b1=[HID], w2=[HID, C], b2=[C],
        w_out=[C, NCLS], b_out=[NCLS],
    )
    nc = bacc.Bacc("TRN2", target_bir_lowering=False, debug=False)
    ins = {k: nc.dram_tensor(k, shapes[k], mybir.dt.float32,
                             kind="ExternalInput").ap()
           for k in _IN_NAMES}
    out_ap = nc.dram_tensor("out", [Tq, NCLS], mybir.dt.float32,
                            kind="ExternalOutput").ap()
    with tile.TileContext(nc) as tc:
        emit_block(tc, out_ap, ins, cfg)
    nc.finalize()
    return nc


_NC_CACHE = {}


def kernel(**inputs) -> np.ndarray:
    from concourse.bass_utils import run_bass_kernel_spmd

    cfg = CFG_FULL
    B, N = cfg["B"], cfg["N"]
    Tq, NCLS = cfg["Tq"], cfg["NCLS"]
    n_cores = 8
    halves = N // Tq  # 2

    if "full" not in _NC_CACHE:
        _NC_CACHE["full"] = _build_nc(cfg)
    nc = _NC_CACHE["full"]

    x = np.ascontiguousarray(np.asarray(inputs["x"], dtype=np.float32))
    value = np.ascontiguousarray(np.asarray(inputs["value"], dtype=np.float32))
    shared = {k: np.ascontiguousarray(np.asarray(inputs[k], dtype=np.float32))
              for k in _IN_NAMES if k not in ("xs", "vals")}

    in_maps = []
    for core in range(n_cores):
        b, hf = core // halves, core % halves
        m = dict(shared)
        m["xs"] = np.ascontiguousarray(np.roll(x[b], -hf * Tq, axis=0))
        m["vals"] = np.ascontiguousarray(np.roll(value[b], -hf * Tq, axis=0))
        in_maps.append(m)

    res = run_bass_kernel_spmd(nc, in_maps, core_ids=list(range(n_cores)))
    out = np.empty((B, N, NCLS), dtype=np.float32)
    for core in range(n_cores):
        b, hf = core // halves, core % halves
        out[b, hf * Tq:(hf + 1) * Tq, :] = res.results[core]["out"]
    return out
